# revision 21
# baseline (speedup 1.0000x reference)
"""Trainium2 Bass kernel for nn_GAT_34059090657327 (6-layer GAT + JKN + attention pooling).

Distribution (8 NeuronCores, SPMD):
  - Nodes dst-sharded: core c owns nodes [2500c, 2500(c+1)), padded to 2560 (20 groups of 128).
  - Edges live on the core owning their dst. Edge slots are dst-major: lane p of group g
    holds up to M in-edges of node g*128+p (along the free dim), overflow edges go to one
    spill chunk per group. With this layout the weighted scatter-add is a PSUM-accumulated
    matmul with an *identity* stationary operand (plus one one-hot matmul for the spill
    chunk), a_d broadcasts per-partition, and the aggregation lands node-major so the
    softmax division is a per-partition scalar op.
  - Per layer: h|a_s|a_d computed for owned nodes (feature-major matmuls), AllGathered into
    a replicated bf16 [20480, 128] DRAM table (row = [h(64)|a_s|a_d|pad]); per-edge rows
    fetched by src via gpsimd dma_gather (256B bf16 descriptors; trailing empty spill-dst
    slots are -1 so the ucode trims their descriptors); attention exp/normalize on-chip in
    f32 (a_s columns cast out of the bf16 gather buffer first).
  - JKN argmax is folded into each layer's MLP slab loop (running max of eps-weighted
    squared norms + masked feature update), removing the serial post-loop JKN pass.
  - Host-side prep is index/weight-only: edge partitioning + packing, the folded edge-
    attention table t = emb @ conv_We @ att_edge gathered per-edge, its per-node mean
    (self-loop attr), and conv bias folded into mlp b1. All x/h-dependent math is on-device.
  - Softmax max-subtraction dropped (logits are O(0.3) for this model; validated exact).
  - JKN argmax via eps-perturbed squared norms; per-graph pooling is core-local (graph
    boundaries align with the node sharding); final MLP -> [8] per core, host concat.
"""
import numpy as np
import sys

sys.path.insert(0, '/opt/trn_rl_repo')

import concourse.bass as bass
import concourse.mybir as mybir
import concourse.tile as tile
from concourse import library_config
from concourse.bass import AP
from concourse.bass_utils import run_bass_kernel_spmd
from concourse.library_overlay import lower_extended_insts
from concourse.tile_rust import add_dep_helper

F32 = mybir.dt.float32
BF16 = mybir.dt.bfloat16
I16 = mybir.dt.int16
OP = mybir.AluOpType
ACTF = mybir.ActivationFunctionType

N, E, NG, DIM, HID, L = 20000, 320000, 64, 128, 64, 6
NC = 8
NPC = N // NC            # 2500
P = 128
GRP = 20                 # node groups of 128 per core
NPAD = GRP * P           # 2560
LRELU = 0.2
GBOUND = [int(np.ceil(j * NPC / 8)) for j in range(9)]  # local graph boundaries

_cache = {}

# ---------------------------------------------------------------------------
# This walrus build encodes only ONE semaphore wait/update per TPB_CTRL
# instruction ("Too many sync wait commands" on the Tile tail drain). Split
# extra waits onto preceding NoOps at BIR-serialization time.
import json as _json


def _fix_prep_sems(j: dict) -> None:
    """Point each SWDGE prep's DMA-completion sem at the Tile DMASW lane sem
    its consumers wait on. Tile assigns gen_mode=1 preps round-robin to the 8
    DMASW lanes (pass 1) and emits consumer waits against those lane sems, but
    leaves the user-passed `sem=` on the prep — so nothing ever increments the
    lane sems. Rewrite on_update[0] of the k-th prep (program order) to lane
    k%8's sem."""
    lanes = {}
    import re
    for fn in j["functions"]:
        for bb in fn["blocks"]:
            for inst in bb["instructions"]:
                si = inst.get("sync_info") or {}
                for w in (si.get("on_wait") or []) + (si.get("on_update") or []):
                    m = re.match(r"DMASW(\d+)_", w.get("ant_name", ""))
                    if m:
                        lanes[int(m.group(1))] = (w["ant_name"], w["id"])
    if not lanes:
        return
    nl = max(lanes) + 1
    assert sorted(lanes) == list(range(nl)), lanes
    k = 0
    for fn in j["functions"]:
        for bb in fn["blocks"]:
            for inst in bb["instructions"]:
                if inst["opcode"] != "DMAGatherAnt":
                    continue
                ups = (inst.get("sync_info") or {}).get("on_update") or []
                if ups and ups[0].get("ant_name") == "gsem":
                    name, sid = lanes[k % nl]
                    ups[0]["ant_name"] = name
                    ups[0]["id"] = sid
                    k += 1


def _split_multiwaits(js: bytes) -> bytes:
    j = _json.loads(js)
    _fix_prep_sems(j)
    n = 0
    for fn in j["functions"]:
        for bb in fn["blocks"]:
            out = []
            for inst in bb["instructions"]:
                si = inst.get("sync_info") or {}
                waits = si.get("on_wait") or []
                if len(waits) > 1:
                    for w in waits[:-1]:
                        n += 1
                        out.append({
                            "name": inst["name"] + f"_w{n}", "opcode": "NoOp",
                            "engine": inst["engine"], "ins": [], "outs": [],
                            "sync_info": {"on_wait": [w], "on_update": []},
                        })
                    si["on_wait"] = [waits[-1]]
                out.append(inst)
                ups = si.get("on_update") or []
                if len(ups) > 1 and inst["opcode"] in ("NoOp", "Drain", "EventSemaphore"):
                    si["on_update"] = [ups[0]]
                    for u in ups[1:]:
                        n += 1
                        out.append({
                            "name": inst["name"] + f"_u{n}", "opcode": "NoOp",
                            "engine": inst["engine"], "ins": [], "outs": [],
                            "sync_info": {"on_wait": [], "on_update": [u]},
                        })
            bb["instructions"] = out
    return _json.dumps(j).encode()


if not getattr(bass.Bass, "_mw_patched", False):
    _orig_to_json_bytes = bass.Bass.to_json_bytes

    def _to_json_bytes_patched(self, *a, **k):
        return _split_multiwaits(_orig_to_json_bytes(self, *a, **k))

    bass.Bass.to_json_bytes = _to_json_bytes_patched
    bass.Bass._mw_patched = True


def _bc(ap, pos, count):
    """Insert a stride-0 (broadcast) dim of `count` at free-dim position `pos`."""
    lst = [list(x) for x in ap.ap]
    lst.insert(1 + pos, [0, count])
    return AP(ap.tensor, ap.offset, lst)


def _build(M):
    CG = M + 2               # chunks per group: M main + spill + spill-dst
    C = GRP * CG             # total chunks per core
    NIG = CG * P             # gather indices per group

    nc = bass.Bass(num_devices=NC)

    # ---------------- inputs ----------------
    e_idxw = nc.dram_tensor("e_idxw", [P, C * 8], I16, kind="ExternalInput")
    e_ae = nc.dram_tensor("e_ae", [P, C, 7], F32, kind="ExternalInput")
    e_sdstl = nc.dram_tensor("e_sdstl", [P, GRP], F32, kind="ExternalInput")
    e_aeloop = nc.dram_tensor("e_aeloop", [P, GRP, L], F32, kind="ExternalInput")
    e_xidx = nc.dram_tensor("e_xidx", [NPAD], F32, kind="ExternalInput")
    w_iota = nc.dram_tensor("w_iota", [P, P], F32, kind="ExternalInput")
    w_iotac = nc.dram_tensor("w_iotac", [P, 1], F32, kind="ExternalInput")
    w_ident = nc.dram_tensor("w_ident", [P, P], F32, kind="ExternalInput")
    w_emb = nc.dram_tensor("w_emb", [P, P], F32, kind="ExternalInput")
    w_conv = nc.dram_tensor("w_conv", [P, L * HID], F32, kind="ExternalInput")
    w_att = nc.dram_tensor("w_att", [HID, L * 2], F32, kind="ExternalInput")
    w_m1 = nc.dram_tensor("w_m1", [HID, L * HID], F32, kind="ExternalInput")
    w_m2 = nc.dram_tensor("w_m2", [HID, L * HID], F32, kind="ExternalInput")
    w_m3 = nc.dram_tensor("w_m3", [HID, L * DIM], F32, kind="ExternalInput")
    w_b1 = nc.dram_tensor("w_b1", [HID, L], F32, kind="ExternalInput")
    w_b2 = nc.dram_tensor("w_b2", [HID, L], F32, kind="ExternalInput")
    w_b3 = nc.dram_tensor("w_b3", [DIM, L], F32, kind="ExternalInput")
    w_eps = nc.dram_tensor("w_eps", [P, L], F32, kind="ExternalInput")
    w_g1w1 = nc.dram_tensor("w_g1w1", [DIM, HID], F32, kind="ExternalInput")
    w_g1b1 = nc.dram_tensor("w_g1b1", [HID, 1], F32, kind="ExternalInput")
    w_g1w2 = nc.dram_tensor("w_g1w2", [HID, 1], F32, kind="ExternalInput")
    w_g1b2 = nc.dram_tensor("w_g1b2", [1, 1], F32, kind="ExternalInput")
    w_g2w1 = nc.dram_tensor("w_g2w1", [DIM, HID], F32, kind="ExternalInput")
    w_g2b1 = nc.dram_tensor("w_g2b1", [HID, 1], F32, kind="ExternalInput")
    w_g2w2 = nc.dram_tensor("w_g2w2", [HID, DIM], F32, kind="ExternalInput")
    w_g2b2 = nc.dram_tensor("w_g2b2", [DIM, 1], F32, kind="ExternalInput")
    w_pw1 = nc.dram_tensor("w_pw1", [DIM, HID], F32, kind="ExternalInput")
    w_pb1 = nc.dram_tensor("w_pb1", [HID, 1], F32, kind="ExternalInput")
    w_pw2 = nc.dram_tensor("w_pw2", [HID, HID], F32, kind="ExternalInput")
    w_pb2 = nc.dram_tensor("w_pb2", [HID, 1], F32, kind="ExternalInput")
    w_pw3 = nc.dram_tensor("w_pw3", [HID, 1], F32, kind="ExternalInput")
    w_pb3 = nc.dram_tensor("w_pb3", [1, 1], F32, kind="ExternalInput")
    eout = nc.dram_tensor("out", [1, 8], F32, kind="ExternalOutput")

    with tile.TileContext(nc) as tc:
        with tc.tile_pool(name="c1", bufs=1) as c1, \
             tc.tile_pool(name="big", bufs=2) as bigp, \
             tc.tile_pool(name="gp", bufs=2) as gp, \
             tc.tile_pool(name="zp", bufs=2) as zp, \
             tc.tile_pool(name="sm", bufs=3) as sm, \
             tc.tile_pool(name="stg", bufs=2) as stgp, \
             tc.tile_pool(name="yp", bufs=2) as yp, \
             tc.tile_pool(name="dr", bufs=1, space="DRAM") as dr:
            ps_stack = tc.tile_pool(name="psA", bufs=4, space="PSUM")
            psA = ps_stack.__enter__()
            ps_stackB = tc.tile_pool(name="psB", bufs=3, space="PSUM")
            psB = ps_stackB.__enter__()

            rel = nc.gpsimd.load_library(library_config.mlp)
            nig_reg = nc.gpsimd.to_reg(CG * P)

            def load(t, shape, tag, dtype=F32):
                s = c1.tile(shape, dtype, tag=tag)
                nc.sync.dma_start(out=s[:], in_=t[:])
                return s

            ident = load(w_ident, [P, P], "ident")
            iota_f = load(w_iota, [P, P], "iota_f")
            iotac = load(w_iotac, [P, 1], "iotac")
            Wconv = load(w_conv, [P, L * HID], "Wconv")
            Watt = load(w_att, [HID, L * 2], "Watt")
            Wm1 = load(w_m1, [HID, L * HID], "Wm1")
            Wm2 = load(w_m2, [HID, L * HID], "Wm2")
            Wm3 = load(w_m3, [HID, L * DIM], "Wm3")
            B1 = load(w_b1, [HID, L], "B1")
            B2 = load(w_b2, [HID, L], "B2")
            B3 = load(w_b3, [DIM, L], "B3")
            Emb = load(w_emb, [P, P], "Emb")
            Eps = load(w_eps, [P, L], "Eps")
            G1W1 = load(w_g1w1, [DIM, HID], "G1W1")
            G1B1 = load(w_g1b1, [HID, 1], "G1B1")
            G1W2 = load(w_g1w2, [HID, 1], "G1W2")
            G1B2 = load(w_g1b2, [1, 1], "G1B2")
            G2W1 = load(w_g2w1, [DIM, HID], "G2W1")
            G2B1 = load(w_g2b1, [HID, 1], "G2B1")
            G2W2 = load(w_g2w2, [HID, DIM], "G2W2")
            G2B2 = load(w_g2b2, [DIM, 1], "G2B2")
            PW1 = load(w_pw1, [DIM, HID], "PW1")
            PB1 = load(w_pb1, [HID, 1], "PB1")
            PW2 = load(w_pw2, [HID, HID], "PW2")
            PB2 = load(w_pb2, [HID, 1], "PB2")
            PW3 = load(w_pw3, [HID, 1], "PW3")
            PB3 = load(w_pb3, [1, 1], "PB3")

            idxw = load(e_idxw, [P, C * 8], "idxw", dtype=I16)
            AE = load(e_ae, [P, C, 7], "AE")
            sdstl = load(e_sdstl, [P, GRP], "sdstl")
            aeloop = load(e_aeloop, [P, GRP, L], "aeloop")

            ones1_128 = c1.tile([1, P], F32, tag="ones1_128")
            nc.vector.memset(ones1_128[:], 1.0)
            identb = c1.tile([P, P], BF16, tag="identb")
            nc.vector.tensor_copy(out=identb[:], in_=ident[:])
            asdf = c1.tile([P, GRP, 2], F32, tag="asdf")

            # x_idx broadcast to [128, NPAD] (partition-stride-0 DMA read)
            xidxb = bigp.tile([P, NPAD], F32, tag="xbig")
            nc.sync.dma_start(out=xidxb[:], in_=AP(e_xidx, 0, [[0, P], [1, NPAD]]))

            # x tiles (jkn entries) + initial x (feature-major [128 f, node])
            xs = [c1.tile([P, NPAD], F32, tag=f"xs{l}", name=f"xs{l}") for l in range(L)]
            x_init = bigp.tile([P, NPAD], F32, tag="xbig")
            for s in range(5):
                sl = slice(s * 512, (s + 1) * 512)
                ohx = stgp.tile([P, 512], F32, tag="stg", name="ohx")
                nc.vector.tensor_scalar(out=ohx[:], in0=xidxb[:, sl],
                                        scalar1=iotac[:], scalar2=None,
                                        op0=OP.is_equal)
                px = psB.tile([P, 512], F32, tag="psB")
                nc.tensor.matmul(out=px[:], lhsT=Emb[:], rhs=ohx[:], start=True, stop=True)
                nc.vector.tensor_copy(out=x_init[:, sl], in_=px[:])

            outc = c1.tile([HID, NPAD], F32, tag="outc")

            # DRAM comm buffers (Shared tensors allow a single writer -> one pair per layer)
            ag_ins = [dr.tile([NPAD, P], BF16, tag=f"ag_in{l}", name=f"ag_in{l}")
                      for l in range(L)]
            ag_outs = [dr.tile([NC * NPAD, P], BF16, tag=f"ag_out{l}", name=f"ag_out{l}",
                               addr_space="Shared") for l in range(L)]

            feat = bigp.tile([P, NPAD], F32, tag="xbig", name="feat", bufs=2)
            mx = c1.tile([1, NPAD], F32, tag="mx")
            r0 = c1.tile([1, NPAD], F32, tag="r0")

            # =================== layers ===================
            for l in range(L):
                x_cur = x_init if l == 0 else xs[l - 1]

                # ---- h | a_s | a_d for owned nodes; node-major staging -> ag_in
                nm = stgp.tile([P, GRP, P], BF16, tag="nm", bufs=1)
                nc.vector.memset(nm[:, :, 66:P], 0.0)
                for s in range(5):
                    sl = slice(s * 512, (s + 1) * 512)
                    ph = psA.tile([HID, 512], F32, tag="psA")
                    nc.tensor.matmul(out=ph[:], lhsT=Wconv[:, l * HID:(l + 1) * HID],
                                     rhs=x_cur[:, sl], start=True, stop=True)
                    stg = stgp.tile([66, 512], F32, tag="stg")
                    nc.vector.tensor_copy(out=stg[0:HID, :], in_=ph[:])
                    pa = psA.tile([2, 512], F32, tag="psA")
                    nc.tensor.matmul(out=pa[:], lhsT=Watt[:, l * 2:(l + 1) * 2],
                                     rhs=stg[0:HID, :], start=True, stop=True)
                    nc.vector.tensor_copy(out=stg[HID:HID + 2, :], in_=pa[:])
                    for t in range(4):
                        g = s * 4 + t
                        ptr = psA.tile([P, 66], F32, tag="psA")
                        nc.tensor.transpose(out=ptr[:], in_=stg[:, t * 128:(t + 1) * 128],
                                            identity=ident[:66, :66])
                        nc.vector.tensor_copy(out=nm[:, g, 0:66], in_=ptr[:])
                        nc.vector.tensor_copy(out=asdf[:, g, :], in_=ptr[:, 64:66])
                nc.sync.dma_start(out=ag_ins[l][:].rearrange("(g p) c -> p g c", p=P),
                                  in_=nm[:])

                # ---- AllGather the node table
                nc.gpsimd.collective_compute(
                    "AllGather", OP.bypass, replica_groups=[list(range(NC))],
                    ins=[ag_ins[l][:]], outs=[ag_outs[l][:]])

                # ---- self-loop weights, node-major [128, GRP]
                wloop = sm.tile([P, GRP], F32, tag="wloop")
                zt = sm.tile([P, GRP], F32, tag="zt")
                nc.vector.tensor_tensor(out=zt[:], in0=asdf[:, :, 0], in1=asdf[:, :, 1],
                                        op=OP.add)
                nc.vector.tensor_tensor(out=zt[:], in0=zt[:], in1=aeloop[:, :, l],
                                        op=OP.add)
                t2 = sm.tile([P, GRP], F32, tag="zt2")
                nc.vector.tensor_scalar_mul(t2[:], zt[:], LRELU)
                nc.vector.tensor_tensor(out=zt[:], in0=zt[:], in1=t2[:], op=OP.max)
                nc.scalar.activation(wloop[:], zt[:], ACTF.Exp)

                # ---- per-group edge processing
                for g in range(GRP):
                    gs = g * CG
                    Gb = gp.tile([P, CG, P], BF16, tag="Gb", bufs=3)
                    gi = nc.gpsimd.dma_gather(
                        out_ap=Gb[:], in_ap=ag_outs[l][:],
                        idxs_ap=idxw[:, gs * 8:(gs + CG) * 8],
                        num_idxs=NIG, num_idxs_reg=nig_reg, elem_size=P,
                        single_packet=False)
                    add_dep_helper(gi.ins, rel.ins, False, "needs mlp lib")
                    # logits -> w  (main slots 0..M-1, spill slot M)
                    zcp = zp.tile([P, M + 2], F32, tag="zcp")
                    nc.vector.tensor_copy(out=zcp[:, 0:M + 1], in_=Gb[:, 0:M + 1, 64])
                    nc.vector.tensor_copy(out=zcp[:, M + 1:M + 2],
                                          in_=Gb[:, M + 1, 65:66])
                    z = zp.tile([P, M + 1], F32, tag="z")
                    nc.vector.tensor_scalar(out=z[:, 0:M], in0=zcp[:, 0:M],
                                            scalar1=asdf[:, g, 1:2], scalar2=None,
                                            op0=OP.add)
                    nc.vector.tensor_tensor(out=z[:, 0:M], in0=z[:, 0:M],
                                            in1=AE[:, gs:gs + M, l], op=OP.add)
                    nc.vector.tensor_tensor(out=z[:, M:M + 1], in0=zcp[:, M:M + 1],
                                            in1=zcp[:, M + 1:M + 2], op=OP.add)
                    nc.vector.tensor_tensor(out=z[:, M:M + 1], in0=z[:, M:M + 1],
                                            in1=AE[:, gs + M, l:l + 1], op=OP.add)
                    t0 = zp.tile([P, M + 1], F32, tag="t0")
                    nc.vector.tensor_scalar_mul(t0[:], z[:], LRELU)
                    nc.vector.tensor_tensor(out=z[:], in0=z[:], in1=t0[:], op=OP.max)
                    w = zp.tile([P, M + 1], F32, tag="w")
                    nc.scalar.activation(w[:], z[:], ACTF.Exp)
                    nc.vector.tensor_tensor(out=w[:], in0=w[:], in1=AE[:, gs:gs + M + 1, 6],
                                            op=OP.mult)
                    wb = zp.tile([P, M + 1], BF16, tag="wb")
                    nc.vector.tensor_copy(out=wb[:], in_=w[:])
                    # denom column + scale rows by w
                    nc.vector.memset(Gb[:, 0:M + 1, 64:65], 1.0)
                    nc.vector.tensor_tensor(out=Gb[:, 0:M + 1, 0:65],
                                            in0=Gb[:, 0:M + 1, 0:65],
                                            in1=_bc(wb[:], 1, 65), op=OP.mult)
                    # self-loop message
                    smsg = sm.tile([P, 65], BF16, tag="smsg")
                    nc.vector.tensor_scalar(out=smsg[:, 0:64], in0=nm[:, g, 0:64],
                                            scalar1=wloop[:, g:g + 1], scalar2=None,
                                            op0=OP.mult)
                    nc.vector.tensor_copy(out=smsg[:, 64:65], in_=wloop[:, g:g + 1])
                    # spill one-hot
                    oh = sm.tile([P, P], BF16, tag="oh")
                    nc.vector.tensor_scalar(out=oh[:], in0=iota_f[:],
                                            scalar1=sdstl[:, g:g + 1], scalar2=None,
                                            op0=OP.is_equal)
                    # scatter-accumulate (node-major). Spill via one-hot matmul;
                    # the M main chunks sum via an in-place bf16 tree-add on Gb
                    # (contiguous DVE ops). This keeps PE nearly idle in the
                    # gather phase: Tile's Pool/PE port-sharing rule serializes
                    # every gather behind all prior PE work, so the old
                    # 21-matmul chain capped the gather pipeline at depth 1.
                    pg = psA.tile([P, 65], F32, tag="psA")
                    nc.tensor.matmul(out=pg[:], lhsT=oh[:], rhs=Gb[:, M, 0:65],
                                     start=True, stop=True)
                    n_ = M
                    while n_ > 1:
                        h_ = n_ // 2
                        nc.vector.tensor_tensor(out=Gb[:, 0:h_, :],
                                                in0=Gb[:, 0:h_, :],
                                                in1=Gb[:, h_:2 * h_, :], op=OP.add)
                        if n_ % 2:
                            nc.vector.tensor_tensor(out=Gb[:, 0, :], in0=Gb[:, 0, :],
                                                    in1=Gb[:, n_ - 1, :], op=OP.add)
                        n_ = h_
                    nc.vector.tensor_tensor(out=Gb[:, 0, 0:65], in0=Gb[:, 0, 0:65],
                                            in1=smsg[:], op=OP.add)
                    redf = sm.tile([P, 65], F32, tag="redf")
                    nc.vector.tensor_copy(out=redf[:], in_=Gb[:, 0, 0:65])
                    nc.vector.tensor_tensor(out=redf[:], in0=redf[:], in1=pg[:],
                                            op=OP.add)
                    # normalize + transpose to feature-major
                    rec = sm.tile([P, 1], F32, tag="rec")
                    nc.vector.reciprocal(out=rec[:], in_=redf[:, 64:65])
                    onm = sm.tile([P, 64], F32, tag="onm")
                    nc.vector.tensor_scalar(out=onm[:], in0=redf[:, 0:64], scalar1=rec[:],
                                            scalar2=None, op0=OP.mult)
                    ptr2 = psA.tile([64, P], F32, tag="psA")
                    nc.tensor.transpose(out=ptr2[:], in_=onm[:], identity=ident[:])
                    nc.vector.tensor_copy(out=outc[:, g * P:(g + 1) * P], in_=ptr2[:])

                # ---- MLP (feature-major)
                for s in range(5):
                    sl = slice(s * 512, (s + 1) * 512)
                    p1 = psB.tile([HID, 512], F32, tag="psB")
                    nc.tensor.matmul(out=p1[:], lhsT=Wm1[:, l * HID:(l + 1) * HID],
                                     rhs=outc[:, sl], start=True, stop=True)
                    y1 = yp.tile([HID, 512], F32, tag="y", name="y1", bufs=3)
                    nc.scalar.activation(y1[:], p1[:], ACTF.Relu, bias=B1[:, l:l + 1])
                    p2 = psB.tile([HID, 512], F32, tag="psB")
                    nc.tensor.matmul(out=p2[:], lhsT=Wm2[:, l * HID:(l + 1) * HID],
                                     rhs=y1[:], start=True, stop=True)
                    y2 = yp.tile([HID, 512], F32, tag="y", name="y2", bufs=3)
                    nc.scalar.activation(y2[:], p2[:], ACTF.Relu, bias=B2[:, l:l + 1])
                    p3 = psB.tile([P, 512], F32, tag="psB")
                    nc.tensor.matmul(out=p3[:], lhsT=Wm3[:, l * DIM:(l + 1) * DIM],
                                     rhs=y2[:], start=True, stop=True)
                    nc.vector.tensor_scalar(out=xs[l][:, sl], in0=p3[:],
                                            scalar1=B3[:, l:l + 1], scalar2=None,
                                            op0=OP.add)
                    # incremental JKN: track running max eps-weighted sq-norm
                    # and the argmax layer's features
                    sq = sm.tile([P, 512], F32, tag="sq", name="sq", bufs=2)
                    nc.scalar.activation(sq[:], xs[l][:, sl], ACTF.Square)
                    pml = psB.tile([1, 512], F32, tag="psB", name="pml")
                    nc.tensor.matmul(out=pml[:], lhsT=Eps[:, l:l + 1], rhs=sq[:],
                                     start=True, stop=True)
                    if l == 0:
                        nc.vector.tensor_copy(out=mx[0:1, sl], in_=pml[:])
                        nc.vector.tensor_copy(out=feat[:, sl], in_=xs[0][:, sl])
                    else:
                        gt = sm.tile([1, 512], F32, tag="gt", name="gt", bufs=2)
                        nc.vector.tensor_tensor(out=gt[:], in0=pml[:],
                                                in1=mx[0:1, sl], op=OP.is_gt)
                        nc.vector.tensor_tensor(out=mx[0:1, sl], in0=mx[0:1, sl],
                                                in1=pml[:], op=OP.max)
                        pgt = psB.tile([P, 512], F32, tag="psB", name="pgt")
                        nc.tensor.matmul(out=pgt[:], lhsT=ones1_128[:], rhs=gt[:],
                                         start=True, stop=True)
                        df = sm.tile([P, 512], F32, tag="df", name="df", bufs=2)
                        nc.vector.tensor_tensor(out=df[:], in0=xs[l][:, sl],
                                                in1=feat[:, sl], op=OP.subtract)
                        nc.vector.tensor_tensor(out=df[:], in0=df[:], in1=pgt[:],
                                                op=OP.mult)
                        nc.vector.tensor_tensor(out=feat[:, sl], in0=feat[:, sl],
                                                in1=df[:], op=OP.add)

            # layer-phase PSUM pools -> pooling-phase pool
            ps_stackB.__exit__(None, None, None)
            ps_stack.__exit__(None, None, None)
            ps_stackC = tc.tile_pool(name="psC", bufs=2, space="PSUM")
            psC = ps_stackC.__enter__()
            psB = psC  # later phases allocate from psC

            # =================== pooling ===================
            h2T = c1.tile([DIM, NPAD], F32, tag="h2T")
            for s in range(5):
                sl = slice(s * 512, (s + 1) * 512)
                pa1 = psB.tile([HID, 512], F32, tag="psC")
                nc.tensor.matmul(out=pa1[:], lhsT=G1W1[:], rhs=feat[:, sl],
                                 start=True, stop=True)
                r1 = yp.tile([HID, 512], F32, tag="y", name="r1", bufs=3)
                nc.scalar.activation(r1[:], pa1[:], ACTF.Relu, bias=G1B1[:])
                ph1 = psB.tile([1, 512], F32, tag="psC", name="ph1")
                nc.tensor.matmul(out=ph1[:], lhsT=G1W2[:], rhs=r1[:], start=True, stop=True)
                nc.scalar.activation(r0[0:1, sl], ph1[:], ACTF.Exp, bias=G1B2[:])
                pa2 = psB.tile([HID, 512], F32, tag="psC")
                nc.tensor.matmul(out=pa2[:], lhsT=G2W1[:], rhs=feat[:, sl],
                                 start=True, stop=True)
                r2 = yp.tile([HID, 512], F32, tag="y", name="r2", bufs=3)
                nc.scalar.activation(r2[:], pa2[:], ACTF.Relu, bias=G2B1[:])
                ph2 = psB.tile([DIM, 512], F32, tag="psC")
                nc.tensor.matmul(out=ph2[:], lhsT=G2W2[:], rhs=r2[:], start=True, stop=True)
                nc.vector.tensor_scalar(out=h2T[:, sl], in0=ph2[:], scalar1=G2B2[:],
                                        scalar2=None, op0=OP.add)

            
            hg = c1.tile([DIM, 8], F32, tag="hg")
            for j in range(8):
                lo, hi = GBOUND[j], GBOUND[j + 1]
                cnt = hi - lo
                sg = sm.tile([1, 1], F32, tag="sg")
                nc.vector.tensor_reduce(out=sg[:], in_=r0[0:1, lo:hi],
                                        axis=mybir.AxisListType.X, op=OP.add)
                nc.vector.tensor_scalar_mul(sg[:], sg[:], float(cnt))
                rg = sm.tile([1, 1], F32, tag="rg")
                nc.vector.reciprocal(out=rg[:], in_=sg[:])
                nc.vector.tensor_scalar(out=r0[0:1, lo:hi], in0=r0[0:1, lo:hi],
                                        scalar1=rg[:], scalar2=None, op0=OP.mult)
                pw = psB.tile([P, 512], F32, tag="psC")
                nc.tensor.matmul(out=pw[0:P, 0:cnt], lhsT=ones1_128[:],
                                 rhs=r0[0:1, lo:hi], start=True, stop=True)
                wh = yp.tile([DIM, 512], F32, tag="y", name="wh", bufs=3)
                nc.vector.tensor_tensor(out=wh[:, 0:cnt], in0=h2T[:, lo:hi],
                                        in1=pw[0:DIM, 0:cnt], op=OP.mult)
                nc.vector.tensor_reduce(out=hg[:, j:j + 1], in_=wh[:, 0:cnt],
                                        axis=mybir.AxisListType.X, op=OP.add)

            pp1 = psB.tile([HID, 8], F32, tag="psC")
            nc.tensor.matmul(out=pp1[:], lhsT=PW1[:], rhs=hg[:], start=True, stop=True)
            rp1 = sm.tile([HID, 8], F32, tag="rp1")
            nc.scalar.activation(rp1[:], pp1[:], ACTF.Relu, bias=PB1[:])
            pp2 = psB.tile([HID, 8], F32, tag="psC")
            nc.tensor.matmul(out=pp2[:], lhsT=PW2[:], rhs=rp1[:], start=True, stop=True)
            rp2 = sm.tile([HID, 8], F32, tag="rp2")
            nc.scalar.activation(rp2[:], pp2[:], ACTF.Relu, bias=PB2[:])
            pp3 = psB.tile([1, 8], F32, tag="psC")
            nc.tensor.matmul(out=pp3[:], lhsT=PW3[:], rhs=rp2[:], start=True, stop=True)
            ores = sm.tile([1, 8], F32, tag="ores")
            nc.vector.tensor_scalar(out=ores[:], in0=pp3[:], scalar1=PB3[:],
                                    scalar2=None, op0=OP.add)
            nc.sync.dma_start(out=eout[:], in_=ores[:])
            ps_stackC.__exit__(None, None, None)

    lower_extended_insts(nc)
    return nc


def _prep_host(inputs):
    src = np.asarray(inputs['edge_index'][0]).astype(np.int64)
    dst = np.asarray(inputs['edge_index'][1]).astype(np.int64)
    attr = np.asarray(inputs['edge_attr_idx']).astype(np.int64)
    x_idx = np.asarray(inputs['x_idx']).astype(np.int64)
    emb = np.asarray(inputs['emb']).astype(np.float32)

    conv_We = np.asarray(inputs['conv_We'], np.float32)
    att_e = np.asarray(inputs['conv_att_edge'], np.float32)
    V = np.stack([conv_We[l] @ att_e[l] for l in range(L)], 1)    # [128, 6]
    t_all = (emb @ V).astype(np.float32)                          # [128, 6]

    owner = dst // NPC
    srcg = ((src // NPC) * NPAD + src % NPC).astype(np.int64)     # padded global id

    per_core = []
    for c in range(NC):
        m = np.where(owner == c)[0]
        dl = (dst[m] - c * NPC).astype(np.int64)
        order = np.argsort(dl, kind='stable')
        eidx = m[order]
        dl = dl[order]
        # segment starts per node
        counts = np.bincount(dl, minlength=NPC)
        starts = np.zeros(NPC + 1, np.int64)
        np.cumsum(counts, out=starts[1:])
        per_core.append((eidx, dl, counts, starts))

    # choose M: smallest with per-group spill <= 128
    M = 8
    while True:
        ok = True
        for c in range(NC):
            counts = np.zeros(NPAD, np.int64)
            counts[:NPC] = per_core[c][2]
            sp = np.maximum(counts - M, 0).reshape(GRP, P).sum(1)
            if sp.max() > P:
                ok = False
                break
        if ok:
            break
        M += 1

    CG = M + 2
    C = GRP * CG
    cores = []
    for c in range(NC):
        eidx, dl, counts, starts = per_core[c]
        idxflat = np.zeros(C * P, np.int64)
        ae = np.zeros((P, C, 7), np.float32)
        sdstl = np.zeros((P, GRP), np.float32)
        ael = np.zeros((NPC,), np.float32)
        for g in range(GRP):
            base = g * CG
            sp_src, sp_dst, sp_attr, sp_lane = [], [], [], []
            for p in range(P):
                n = g * P + p
                if n >= NPC:
                    continue
                s0, cnt = starts[n], counts[n]
                take = min(cnt, M)
                es = eidx[s0:s0 + cnt]
                for k in range(take):
                    ch = base + k
                    idxflat[ch * P + p] = srcg[es[k]]
                    ae[p, ch, 0:6] = t_all[attr[es[k]]]
                    ae[p, ch, 6] = 1.0
                if cnt > M:
                    for k in range(M, cnt):
                        sp_src.append(srcg[es[k]])
                        sp_dst.append(c * NPAD + n)
                        sp_attr.append(attr[es[k]])
                        sp_lane.append(p)
            ns = len(sp_src)
            assert ns <= P
            chs, chd = base + M, base + M + 1
            for j in range(ns):
                idxflat[chs * P + j] = sp_src[j]
                idxflat[chd * P + j] = sp_dst[j]
                ae[j, chs, 0:6] = t_all[sp_attr[j]]
                ae[j, chs, 6] = 1.0
                sdstl[j, g] = float(sp_lane[j])
            # trailing empty slots of the last (spill-dst) chunk: mark -1 so
            # the gather ucode trims them (descriptors skipped; stale Gb data
            # in those lanes is masked by ae[...,6]=0)
            idxflat[chd * P + ns:(chd + 1) * P] = -1
        # wrapped int16 index layout, replicated per 16-partition group
        NIDX = C * P
        idxw = np.zeros((P, NIDX // 16), np.int16)
        fl = idxflat.astype(np.int16)
        for r in range(16):
            idxw[r::16, :] = fl[r::16].reshape(1, -1)
        # per-node loop attr (host: pure index/weight math)
        ae_sum = np.zeros((NPC, L), np.float32)
        deg = counts.astype(np.float32)
        np.add.at(ae_sum, dl, t_all[attr[eidx]])
        ael = ae_sum / np.maximum(deg, 1.0)[:, None]
        ael_pad = np.zeros((NPAD, L), np.float32)
        ael_pad[:NPC] = ael
        aeloop = ael_pad.reshape(GRP, P, L).transpose(1, 0, 2).copy()
        cores.append(dict(e_idxw=idxw, e_ae=ae, e_sdstl=sdstl, e_aeloop=aeloop))

    # ---- shared weights
    conv_W = np.asarray(inputs['conv_W'], np.float32)
    att_s = np.asarray(inputs['conv_att_src'], np.float32)
    att_d = np.asarray(inputs['conv_att_dst'], np.float32)
    conv_b = np.asarray(inputs['conv_b'], np.float32)
    m1 = np.asarray(inputs['mlp_W1'], np.float32)
    m2 = np.asarray(inputs['mlp_W2'], np.float32)
    m3 = np.asarray(inputs['mlp_W3'], np.float32)
    b1 = np.asarray(inputs['mlp_b1'], np.float32)
    b2 = np.asarray(inputs['mlp_b2'], np.float32)
    b3 = np.asarray(inputs['mlp_b3'], np.float32)
    b1_eff = np.stack([conv_b[l] @ m1[l] + b1[l] for l in range(L)], 1)

    shared = dict(
        w_iota=np.broadcast_to(np.arange(P, dtype=np.float32)[None, :], (P, P)).copy(),
        w_iotac=np.arange(P, dtype=np.float32).reshape(P, 1),
        w_ident=np.eye(P, dtype=np.float32),
        w_emb=emb,
        w_conv=np.concatenate([conv_W[l] for l in range(L)], 1),
        w_att=np.concatenate([np.stack([att_s[l], att_d[l]], 1) for l in range(L)], 1),
        w_m1=np.concatenate([m1[l] for l in range(L)], 1),
        w_m2=np.concatenate([m2[l] for l in range(L)], 1),
        w_m3=np.concatenate([m3[l] for l in range(L)], 1),
        w_b1=b1_eff,
        w_b2=b2.T.copy(),
        w_b3=b3.T.copy(),
        w_eps=np.broadcast_to((1.0 - np.arange(L, dtype=np.float32) * 1e-7)[None, :],
                              (P, L)).copy(),
        w_g1w1=np.asarray(inputs['g1_W1'], np.float32),
        w_g1b1=np.asarray(inputs['g1_b1'], np.float32).reshape(HID, 1),
        w_g1w2=np.asarray(inputs['g1_W2'], np.float32),
        w_g1b2=np.asarray(inputs['g1_b2'], np.float32).reshape(1, 1),
        w_g2w1=np.asarray(inputs['g2_W1'], np.float32),
        w_g2b1=np.asarray(inputs['g2_b1'], np.float32).reshape(HID, 1),
        w_g2w2=np.asarray(inputs['g2_W2'], np.float32),
        w_g2b2=np.asarray(inputs['g2_b2'], np.float32).reshape(DIM, 1),
        w_pw1=np.asarray(inputs['p_W1'], np.float32),
        w_pb1=np.asarray(inputs['p_b1'], np.float32).reshape(HID, 1),
        w_pw2=np.asarray(inputs['p_W2'], np.float32),
        w_pb2=np.asarray(inputs['p_b2'], np.float32).reshape(HID, 1),
        w_pw3=np.asarray(inputs['p_W3'], np.float32),
        w_pb3=np.asarray(inputs['p_b3'], np.float32).reshape(1, 1),
    )

    in_maps = []
    for c in range(NC):
        xi = np.full(NPAD, -1.0, np.float32)
        xi[:NPC] = x_idx[c * NPC:(c + 1) * NPC].astype(np.float32)
        mm = dict(shared)
        mm.update(cores[c])
        mm['e_xidx'] = xi
        in_maps.append(mm)
    return M, in_maps


def kernel(**inputs):
    M, in_maps = _prep_host(inputs)
    if M not in _cache:
        _cache[M] = _build(M)
    nc = _cache[M]
    res = run_bass_kernel_spmd(nc, in_maps, core_ids=list(range(NC)))
    out = np.concatenate([np.asarray(res.results[c]['out']).reshape(8)
                          for c in range(NC)])
    return out.astype(np.float32)


if __name__ == "__main__":
    import jax
    sys.path.insert(0, '/root/problem')
    import reference as R
    with jax.default_device(jax.devices('cpu')[0]):
        inp = R.setup_inputs()
        exp = np.asarray(R.reference(**inp))
    inp = {k: np.asarray(v) for k, v in inp.items()}
    act = kernel(**inp)
    rel = np.linalg.norm(act - exp) / np.linalg.norm(exp)
    print("Relative error:", rel)



# revision 22
# speedup vs baseline: 1.1441x; 1.1441x over previous
"""Trainium2 Bass kernel for nn_GAT_34059090657327 (6-layer GAT + JKN + attention pooling).

Distribution (8 NeuronCores, SPMD):
  - Nodes dst-sharded: core c owns nodes [2500c, 2500(c+1)), padded to 2560 (20 groups of 128).
  - Edges live on the core owning their dst. Edge slots are dst-major: lane p of group g
    holds up to M in-edges of node g*128+p (along the free dim), overflow edges go to one
    spill chunk per group. With this layout the weighted scatter-add is a PSUM-accumulated
    matmul with an *identity* stationary operand (plus one one-hot matmul for the spill
    chunk), a_d broadcasts per-partition, and the aggregation lands node-major so the
    softmax division is a per-partition scalar op.
  - Per layer: h|a_s|a_d computed for owned nodes (feature-major matmuls), AllGathered into
    a replicated bf16 [20480, 128] DRAM table (row = [h(64)|a_s|a_d|pad]); per-edge rows
    fetched by src via gpsimd dma_gather (256B bf16 descriptors; trailing empty spill-dst
    slots are -1 so the ucode trims their descriptors); attention exp/normalize on-chip in
    f32 (a_s columns cast out of the bf16 gather buffer first).
  - JKN argmax is folded into each layer's MLP slab loop (running max of eps-weighted
    squared norms + masked feature update), removing the serial post-loop JKN pass.
  - Host-side prep is index/weight-only: edge partitioning + packing, the folded edge-
    attention table t = emb @ conv_We @ att_edge gathered per-edge, its per-node mean
    (self-loop attr), and conv bias folded into mlp b1. All x/h-dependent math is on-device.
  - Softmax max-subtraction dropped (logits are O(0.3) for this model; validated exact).
  - JKN argmax via eps-perturbed squared norms; per-graph pooling is core-local (graph
    boundaries align with the node sharding); final MLP -> [8] per core, host concat.
"""
import numpy as np
import sys

sys.path.insert(0, '/opt/trn_rl_repo')

import concourse.bass as bass
import concourse.mybir as mybir
import concourse.tile as tile
from concourse import library_config
from concourse.bass import AP
from concourse.bass_utils import run_bass_kernel_spmd
from concourse.library_overlay import lower_extended_insts
from concourse.tile_rust import add_dep_helper

F32 = mybir.dt.float32
BF16 = mybir.dt.bfloat16
I16 = mybir.dt.int16
OP = mybir.AluOpType
ACTF = mybir.ActivationFunctionType

N, E, NG, DIM, HID, L = 20000, 320000, 64, 128, 64, 6
NC = 8
NPC = N // NC            # 2500
P = 128
GRP = 20                 # node groups of 128 per core
NPAD = GRP * P           # 2560
LRELU = 0.2
GBOUND = [int(np.ceil(j * NPC / 8)) for j in range(9)]  # local graph boundaries

_cache = {}

# ---------------------------------------------------------------------------
# This walrus build encodes only ONE semaphore wait/update per TPB_CTRL
# instruction ("Too many sync wait commands" on the Tile tail drain). Split
# extra waits onto preceding NoOps at BIR-serialization time.
import json as _json


def _fix_prep_sems(j: dict) -> None:
    """Point each SWDGE prep's DMA-completion sem at the Tile DMASW lane sem
    its consumers wait on. Tile assigns gen_mode=1 preps round-robin to the 8
    DMASW lanes (pass 1) and emits consumer waits against those lane sems, but
    leaves the user-passed `sem=` on the prep — so nothing ever increments the
    lane sems. Rewrite on_update[0] of the k-th prep (program order) to lane
    k%8's sem."""
    lanes = {}
    import re
    for fn in j["functions"]:
        for bb in fn["blocks"]:
            for inst in bb["instructions"]:
                si = inst.get("sync_info") or {}
                for w in (si.get("on_wait") or []) + (si.get("on_update") or []):
                    m = re.match(r"DMASW(\d+)_", w.get("ant_name", ""))
                    if m:
                        lanes[int(m.group(1))] = (w["ant_name"], w["id"])
    if not lanes:
        return
    nl = max(lanes) + 1
    assert sorted(lanes) == list(range(nl)), lanes
    k = 0
    for fn in j["functions"]:
        for bb in fn["blocks"]:
            for inst in bb["instructions"]:
                if inst["opcode"] != "DMAGatherAnt":
                    continue
                ups = (inst.get("sync_info") or {}).get("on_update") or []
                if ups and ups[0].get("ant_name") == "gsem":
                    name, sid = lanes[k % nl]
                    ups[0]["ant_name"] = name
                    ups[0]["id"] = sid
                    k += 1


def _split_multiwaits(js: bytes) -> bytes:
    j = _json.loads(js)
    _fix_prep_sems(j)
    n = 0
    for fn in j["functions"]:
        for bb in fn["blocks"]:
            out = []
            for inst in bb["instructions"]:
                si = inst.get("sync_info") or {}
                waits = si.get("on_wait") or []
                if len(waits) > 1:
                    for w in waits[:-1]:
                        n += 1
                        out.append({
                            "name": inst["name"] + f"_w{n}", "opcode": "NoOp",
                            "engine": inst["engine"], "ins": [], "outs": [],
                            "sync_info": {"on_wait": [w], "on_update": []},
                        })
                    si["on_wait"] = [waits[-1]]
                out.append(inst)
                ups = si.get("on_update") or []
                if len(ups) > 1 and inst["opcode"] in ("NoOp", "Drain", "EventSemaphore"):
                    si["on_update"] = [ups[0]]
                    for u in ups[1:]:
                        n += 1
                        out.append({
                            "name": inst["name"] + f"_u{n}", "opcode": "NoOp",
                            "engine": inst["engine"], "ins": [], "outs": [],
                            "sync_info": {"on_wait": [], "on_update": [u]},
                        })
            bb["instructions"] = out
    return _json.dumps(j).encode()


if not getattr(bass.Bass, "_mw_patched", False):
    _orig_to_json_bytes = bass.Bass.to_json_bytes

    def _to_json_bytes_patched(self, *a, **k):
        return _split_multiwaits(_orig_to_json_bytes(self, *a, **k))

    bass.Bass.to_json_bytes = _to_json_bytes_patched
    bass.Bass._mw_patched = True


def _bc(ap, pos, count):
    """Insert a stride-0 (broadcast) dim of `count` at free-dim position `pos`."""
    lst = [list(x) for x in ap.ap]
    lst.insert(1 + pos, [0, count])
    return AP(ap.tensor, ap.offset, lst)


def _build(M):
    CG = M + 2               # chunks per group: M main + spill + spill-dst
    C = GRP * CG             # total chunks per core
    NIG = CG * P             # gather indices per group

    nc = bass.Bass(num_devices=NC)

    # ---------------- inputs ----------------
    e_idxw = nc.dram_tensor("e_idxw", [P, C * 8], I16, kind="ExternalInput")
    e_ae = nc.dram_tensor("e_ae", [P, C, 7], F32, kind="ExternalInput")
    e_sdstl = nc.dram_tensor("e_sdstl", [P, GRP], F32, kind="ExternalInput")
    e_aeloop = nc.dram_tensor("e_aeloop", [P, GRP, L], F32, kind="ExternalInput")
    e_xidx = nc.dram_tensor("e_xidx", [NPAD], F32, kind="ExternalInput")
    w_iota = nc.dram_tensor("w_iota", [P, P], F32, kind="ExternalInput")
    w_iotac = nc.dram_tensor("w_iotac", [P, 1], F32, kind="ExternalInput")
    w_ident = nc.dram_tensor("w_ident", [P, P], F32, kind="ExternalInput")
    w_emb = nc.dram_tensor("w_emb", [P, P], F32, kind="ExternalInput")
    w_conv = nc.dram_tensor("w_conv", [P, L * HID], F32, kind="ExternalInput")
    w_att = nc.dram_tensor("w_att", [HID, L * 2], F32, kind="ExternalInput")
    w_m1 = nc.dram_tensor("w_m1", [HID, L * HID], F32, kind="ExternalInput")
    w_m2 = nc.dram_tensor("w_m2", [HID, L * HID], F32, kind="ExternalInput")
    w_m3 = nc.dram_tensor("w_m3", [HID, L * DIM], F32, kind="ExternalInput")
    w_b1 = nc.dram_tensor("w_b1", [HID, L], F32, kind="ExternalInput")
    w_b2 = nc.dram_tensor("w_b2", [HID, L], F32, kind="ExternalInput")
    w_b3 = nc.dram_tensor("w_b3", [DIM, L], F32, kind="ExternalInput")
    w_eps = nc.dram_tensor("w_eps", [P, L], F32, kind="ExternalInput")
    w_g1w1 = nc.dram_tensor("w_g1w1", [DIM, HID], F32, kind="ExternalInput")
    w_g1b1 = nc.dram_tensor("w_g1b1", [HID, 1], F32, kind="ExternalInput")
    w_g1w2 = nc.dram_tensor("w_g1w2", [HID, 1], F32, kind="ExternalInput")
    w_g1b2 = nc.dram_tensor("w_g1b2", [1, 1], F32, kind="ExternalInput")
    w_g2w1 = nc.dram_tensor("w_g2w1", [DIM, HID], F32, kind="ExternalInput")
    w_g2b1 = nc.dram_tensor("w_g2b1", [HID, 1], F32, kind="ExternalInput")
    w_g2w2 = nc.dram_tensor("w_g2w2", [HID, DIM], F32, kind="ExternalInput")
    w_g2b2 = nc.dram_tensor("w_g2b2", [DIM, 1], F32, kind="ExternalInput")
    w_pw1 = nc.dram_tensor("w_pw1", [DIM, HID], F32, kind="ExternalInput")
    w_pb1 = nc.dram_tensor("w_pb1", [HID, 1], F32, kind="ExternalInput")
    w_pw2 = nc.dram_tensor("w_pw2", [HID, HID], F32, kind="ExternalInput")
    w_pb2 = nc.dram_tensor("w_pb2", [HID, 1], F32, kind="ExternalInput")
    w_pw3 = nc.dram_tensor("w_pw3", [HID, 1], F32, kind="ExternalInput")
    w_pb3 = nc.dram_tensor("w_pb3", [1, 1], F32, kind="ExternalInput")
    eout = nc.dram_tensor("out", [1, 8], F32, kind="ExternalOutput")

    with tile.TileContext(nc) as tc:
        with tc.tile_pool(name="c1", bufs=1) as c1, \
             tc.tile_pool(name="big", bufs=2) as bigp, \
             tc.tile_pool(name="gp", bufs=2) as gp, \
             tc.tile_pool(name="zp", bufs=2) as zp, \
             tc.tile_pool(name="sm", bufs=3) as sm, \
             tc.tile_pool(name="stg", bufs=2) as stgp, \
             tc.tile_pool(name="yp", bufs=2) as yp, \
             tc.tile_pool(name="dr", bufs=1, space="DRAM") as dr:
            ps_stack = tc.tile_pool(name="psA", bufs=4, space="PSUM")
            psA = ps_stack.__enter__()
            ps_stackB = tc.tile_pool(name="psB", bufs=3, space="PSUM")
            psB = ps_stackB.__enter__()

            rel = nc.gpsimd.load_library(library_config.mlp)
            nig_reg = nc.gpsimd.to_reg(CG * P)

            def load(t, shape, tag, dtype=F32):
                s = c1.tile(shape, dtype, tag=tag)
                nc.sync.dma_start(out=s[:], in_=t[:])
                return s

            ident = load(w_ident, [P, P], "ident")
            iota_f = load(w_iota, [P, P], "iota_f")
            iotac = load(w_iotac, [P, 1], "iotac")
            Wconv = load(w_conv, [P, L * HID], "Wconv")
            Watt = load(w_att, [HID, L * 2], "Watt")
            Wm1 = load(w_m1, [HID, L * HID], "Wm1")
            Wm2 = load(w_m2, [HID, L * HID], "Wm2")
            Wm3 = load(w_m3, [HID, L * DIM], "Wm3")
            B1 = load(w_b1, [HID, L], "B1")
            B2 = load(w_b2, [HID, L], "B2")
            B3 = load(w_b3, [DIM, L], "B3")
            Emb = load(w_emb, [P, P], "Emb")
            Eps = load(w_eps, [P, L], "Eps")
            G1W1 = load(w_g1w1, [DIM, HID], "G1W1")
            G1B1 = load(w_g1b1, [HID, 1], "G1B1")
            G1W2 = load(w_g1w2, [HID, 1], "G1W2")
            G1B2 = load(w_g1b2, [1, 1], "G1B2")
            G2W1 = load(w_g2w1, [DIM, HID], "G2W1")
            G2B1 = load(w_g2b1, [HID, 1], "G2B1")
            G2W2 = load(w_g2w2, [HID, DIM], "G2W2")
            G2B2 = load(w_g2b2, [DIM, 1], "G2B2")
            PW1 = load(w_pw1, [DIM, HID], "PW1")
            PB1 = load(w_pb1, [HID, 1], "PB1")
            PW2 = load(w_pw2, [HID, HID], "PW2")
            PB2 = load(w_pb2, [HID, 1], "PB2")
            PW3 = load(w_pw3, [HID, 1], "PW3")
            PB3 = load(w_pb3, [1, 1], "PB3")

            idxw = load(e_idxw, [P, C * 8], "idxw", dtype=I16)
            AE = load(e_ae, [P, C, 7], "AE")
            sdstl = load(e_sdstl, [P, GRP], "sdstl")
            aeloop = load(e_aeloop, [P, GRP, L], "aeloop")

            ones1_128 = c1.tile([1, P], F32, tag="ones1_128")
            nc.vector.memset(ones1_128[:], 1.0)
            identb = c1.tile([P, P], BF16, tag="identb")
            nc.vector.tensor_copy(out=identb[:], in_=ident[:])
            asdf = c1.tile([P, GRP, 2], F32, tag="asdf")

            # x_idx broadcast to [128, NPAD] (partition-stride-0 DMA read)
            xidxb = bigp.tile([P, NPAD], F32, tag="xbig")
            nc.sync.dma_start(out=xidxb[:], in_=AP(e_xidx, 0, [[0, P], [1, NPAD]]))

            # x tiles (jkn entries) + initial x (feature-major [128 f, node])
            xs = [c1.tile([P, NPAD], F32, tag=f"xs{l}", name=f"xs{l}") for l in range(L)]
            x_init = bigp.tile([P, NPAD], F32, tag="xbig")
            for s in range(5):
                sl = slice(s * 512, (s + 1) * 512)
                ohx = stgp.tile([P, 512], F32, tag="stg", name="ohx")
                nc.vector.tensor_scalar(out=ohx[:], in0=xidxb[:, sl],
                                        scalar1=iotac[:], scalar2=None,
                                        op0=OP.is_equal)
                px = psB.tile([P, 512], F32, tag="psB")
                nc.tensor.matmul(out=px[:], lhsT=Emb[:], rhs=ohx[:], start=True, stop=True)
                nc.vector.tensor_copy(out=x_init[:, sl], in_=px[:])

            outc = c1.tile([HID, NPAD], F32, tag="outc")

            # DRAM comm buffers (Shared tensors allow a single writer -> one pair per layer)
            ag_ins = [dr.tile([NPAD, P], BF16, tag=f"ag_in{l}", name=f"ag_in{l}")
                      for l in range(L)]
            ag_outs = [dr.tile([NC * NPAD, P], BF16, tag=f"ag_out{l}", name=f"ag_out{l}",
                               addr_space="Shared") for l in range(L)]

            feat = bigp.tile([P, NPAD], F32, tag="xbig", name="feat", bufs=2)
            mx = c1.tile([1, NPAD], F32, tag="mx")
            r0 = c1.tile([1, NPAD], F32, tag="r0")

            # =================== layers ===================
            for l in range(L):
                x_cur = x_init if l == 0 else xs[l - 1]

                # ---- h | a_s | a_d for owned nodes; node-major staging -> ag_in
                nm = stgp.tile([P, GRP, P], BF16, tag="nm", bufs=1)
                nc.vector.memset(nm[:, :, 66:P], 0.0)
                for s in range(5):
                    sl = slice(s * 512, (s + 1) * 512)
                    ph = psA.tile([HID, 512], F32, tag="psA")
                    nc.tensor.matmul(out=ph[:], lhsT=Wconv[:, l * HID:(l + 1) * HID],
                                     rhs=x_cur[:, sl], start=True, stop=True)
                    stg = stgp.tile([66, 512], F32, tag="stg")
                    nc.vector.tensor_copy(out=stg[0:HID, :], in_=ph[:])
                    pa = psA.tile([2, 512], F32, tag="psA")
                    nc.tensor.matmul(out=pa[:], lhsT=Watt[:, l * 2:(l + 1) * 2],
                                     rhs=stg[0:HID, :], start=True, stop=True)
                    nc.vector.tensor_copy(out=stg[HID:HID + 2, :], in_=pa[:])
                    for t in range(4):
                        g = s * 4 + t
                        ptr = psA.tile([P, 66], F32, tag="psA")
                        nc.tensor.transpose(out=ptr[:], in_=stg[:, t * 128:(t + 1) * 128],
                                            identity=ident[:66, :66])
                        nc.vector.tensor_copy(out=nm[:, g, 0:66], in_=ptr[:])
                        nc.vector.tensor_copy(out=asdf[:, g, :], in_=ptr[:, 64:66])
                nc.sync.dma_start(out=ag_ins[l][:].rearrange("(g p) c -> p g c", p=P),
                                  in_=nm[:])

                # ---- AllGather the node table
                nc.gpsimd.collective_compute(
                    "AllGather", OP.bypass, replica_groups=[list(range(NC))],
                    ins=[ag_ins[l][:]], outs=[ag_outs[l][:]])

                # ---- self-loop weights, node-major [128, GRP]
                wloop = sm.tile([P, GRP], F32, tag="wloop")
                zt = sm.tile([P, GRP], F32, tag="zt")
                nc.vector.tensor_tensor(out=zt[:], in0=asdf[:, :, 0], in1=asdf[:, :, 1],
                                        op=OP.add)
                nc.vector.tensor_tensor(out=zt[:], in0=zt[:], in1=aeloop[:, :, l],
                                        op=OP.add)
                t2 = sm.tile([P, GRP], F32, tag="zt2")
                nc.vector.tensor_scalar_mul(t2[:], zt[:], LRELU)
                nc.vector.tensor_tensor(out=zt[:], in0=zt[:], in1=t2[:], op=OP.max)
                nc.scalar.activation(wloop[:], zt[:], ACTF.Exp)

                # ---- per-group edge processing
                for g in range(GRP):
                    gs = g * CG
                    Gb = gp.tile([P, CG, P], BF16, tag="Gb", bufs=3)
                    gi = nc.gpsimd.dma_gather(
                        out_ap=Gb[:], in_ap=ag_outs[l][:],
                        idxs_ap=idxw[:, gs * 8:(gs + CG) * 8],
                        num_idxs=NIG, num_idxs_reg=nig_reg, elem_size=P,
                        single_packet=False)
                    add_dep_helper(gi.ins, rel.ins, False, "needs mlp lib")
                    # logits -> w  (main slots 0..M-1, spill slot M)
                    zcp = zp.tile([P, M + 2], F32, tag="zcp")
                    nc.vector.tensor_copy(out=zcp[:, 0:M + 1], in_=Gb[:, 0:M + 1, 64])
                    nc.vector.tensor_copy(out=zcp[:, M + 1:M + 2],
                                          in_=Gb[:, M + 1, 65:66])
                    z = zp.tile([P, M + 1], F32, tag="z")
                    nc.vector.tensor_scalar(out=z[:, 0:M], in0=zcp[:, 0:M],
                                            scalar1=asdf[:, g, 1:2], scalar2=None,
                                            op0=OP.add)
                    nc.vector.tensor_tensor(out=z[:, 0:M], in0=z[:, 0:M],
                                            in1=AE[:, gs:gs + M, l], op=OP.add)
                    nc.vector.tensor_tensor(out=z[:, M:M + 1], in0=zcp[:, M:M + 1],
                                            in1=zcp[:, M + 1:M + 2], op=OP.add)
                    nc.vector.tensor_tensor(out=z[:, M:M + 1], in0=z[:, M:M + 1],
                                            in1=AE[:, gs + M, l:l + 1], op=OP.add)
                    t0 = zp.tile([P, M + 1], F32, tag="t0")
                    nc.vector.tensor_scalar_mul(t0[:], z[:], LRELU)
                    nc.vector.tensor_tensor(out=z[:], in0=z[:], in1=t0[:], op=OP.max)
                    w = zp.tile([P, M + 1], F32, tag="w")
                    nc.scalar.activation(w[:], z[:], ACTF.Exp)
                    nc.vector.tensor_tensor(out=w[:], in0=w[:], in1=AE[:, gs:gs + M + 1, 6],
                                            op=OP.mult)
                    wb = zp.tile([P, M + 1], BF16, tag="wb")
                    nc.vector.tensor_copy(out=wb[:], in_=w[:])
                    # denom column + scale rows by w
                    nc.vector.memset(Gb[:, 0:M + 1, 64:65], 1.0)
                    nc.vector.tensor_tensor(out=Gb[:, 0:M + 1, 0:65],
                                            in0=Gb[:, 0:M + 1, 0:65],
                                            in1=_bc(wb[:], 1, 65), op=OP.mult)
                    # self-loop message
                    smsg = sm.tile([P, 65], BF16, tag="smsg")
                    nc.vector.tensor_scalar(out=smsg[:, 0:64], in0=nm[:, g, 0:64],
                                            scalar1=wloop[:, g:g + 1], scalar2=None,
                                            op0=OP.mult)
                    nc.vector.tensor_copy(out=smsg[:, 64:65], in_=wloop[:, g:g + 1])
                    # spill one-hot
                    oh = sm.tile([P, P], BF16, tag="oh")
                    nc.vector.tensor_scalar(out=oh[:], in0=iota_f[:],
                                            scalar1=sdstl[:, g:g + 1], scalar2=None,
                                            op0=OP.is_equal)
                    # scatter-accumulate (node-major)
                    pg = psA.tile([P, 65], F32, tag="psA")
                    for k in range(M):
                        nc.tensor.matmul(out=pg[:], lhsT=identb[:], rhs=Gb[:, k, 0:65],
                                         start=(k == 0), stop=False)
                    nc.tensor.matmul(out=pg[:], lhsT=identb[:], rhs=smsg[:],
                                     start=False, stop=False)
                    nc.tensor.matmul(out=pg[:], lhsT=oh[:], rhs=Gb[:, M, 0:65],
                                     start=False, stop=True)
                    # normalize + transpose to feature-major
                    rec = sm.tile([P, 1], F32, tag="rec")
                    nc.vector.reciprocal(out=rec[:], in_=pg[:, 64:65])
                    onm = sm.tile([P, 64], F32, tag="onm")
                    nc.vector.tensor_scalar(out=onm[:], in0=pg[:, 0:64], scalar1=rec[:],
                                            scalar2=None, op0=OP.mult)
                    ptr2 = psA.tile([64, P], F32, tag="psA")
                    nc.tensor.transpose(out=ptr2[:], in_=onm[:], identity=ident[:])
                    nc.vector.tensor_copy(out=outc[:, g * P:(g + 1) * P], in_=ptr2[:])

                # ---- MLP (feature-major)
                for s in range(5):
                    sl = slice(s * 512, (s + 1) * 512)
                    p1 = psB.tile([HID, 512], F32, tag="psB")
                    nc.tensor.matmul(out=p1[:], lhsT=Wm1[:, l * HID:(l + 1) * HID],
                                     rhs=outc[:, sl], start=True, stop=True)
                    y1 = yp.tile([HID, 512], F32, tag="y", name="y1", bufs=3)
                    nc.scalar.activation(y1[:], p1[:], ACTF.Relu, bias=B1[:, l:l + 1])
                    p2 = psB.tile([HID, 512], F32, tag="psB")
                    nc.tensor.matmul(out=p2[:], lhsT=Wm2[:, l * HID:(l + 1) * HID],
                                     rhs=y1[:], start=True, stop=True)
                    y2 = yp.tile([HID, 512], F32, tag="y", name="y2", bufs=3)
                    nc.scalar.activation(y2[:], p2[:], ACTF.Relu, bias=B2[:, l:l + 1])
                    p3 = psB.tile([P, 512], F32, tag="psB")
                    nc.tensor.matmul(out=p3[:], lhsT=Wm3[:, l * DIM:(l + 1) * DIM],
                                     rhs=y2[:], start=True, stop=True)
                    nc.vector.tensor_scalar(out=xs[l][:, sl], in0=p3[:],
                                            scalar1=B3[:, l:l + 1], scalar2=None,
                                            op0=OP.add)
                    # incremental JKN: track running max eps-weighted sq-norm
                    # and the argmax layer's features
                    sq = sm.tile([P, 512], F32, tag="sq", name="sq", bufs=2)
                    nc.scalar.activation(sq[:], xs[l][:, sl], ACTF.Square)
                    pml = psB.tile([1, 512], F32, tag="psB", name="pml")
                    nc.tensor.matmul(out=pml[:], lhsT=Eps[:, l:l + 1], rhs=sq[:],
                                     start=True, stop=True)
                    if l == 0:
                        nc.vector.tensor_copy(out=mx[0:1, sl], in_=pml[:])
                        nc.vector.tensor_copy(out=feat[:, sl], in_=xs[0][:, sl])
                    else:
                        gt = sm.tile([1, 512], F32, tag="gt", name="gt", bufs=2)
                        nc.vector.tensor_tensor(out=gt[:], in0=pml[:],
                                                in1=mx[0:1, sl], op=OP.is_gt)
                        nc.vector.tensor_tensor(out=mx[0:1, sl], in0=mx[0:1, sl],
                                                in1=pml[:], op=OP.max)
                        pgt = psB.tile([P, 512], F32, tag="psB", name="pgt")
                        nc.tensor.matmul(out=pgt[:], lhsT=ones1_128[:], rhs=gt[:],
                                         start=True, stop=True)
                        df = sm.tile([P, 512], F32, tag="df", name="df", bufs=2)
                        nc.vector.tensor_tensor(out=df[:], in0=xs[l][:, sl],
                                                in1=feat[:, sl], op=OP.subtract)
                        nc.vector.tensor_tensor(out=df[:], in0=df[:], in1=pgt[:],
                                                op=OP.mult)
                        nc.vector.tensor_tensor(out=feat[:, sl], in0=feat[:, sl],
                                                in1=df[:], op=OP.add)

            # layer-phase PSUM pools -> pooling-phase pool
            ps_stackB.__exit__(None, None, None)
            ps_stack.__exit__(None, None, None)
            ps_stackC = tc.tile_pool(name="psC", bufs=2, space="PSUM")
            psC = ps_stackC.__enter__()
            psB = psC  # later phases allocate from psC

            # =================== pooling ===================
            h2T = c1.tile([DIM, NPAD], F32, tag="h2T")
            for s in range(5):
                sl = slice(s * 512, (s + 1) * 512)
                pa1 = psB.tile([HID, 512], F32, tag="psC")
                nc.tensor.matmul(out=pa1[:], lhsT=G1W1[:], rhs=feat[:, sl],
                                 start=True, stop=True)
                r1 = yp.tile([HID, 512], F32, tag="y", name="r1", bufs=3)
                nc.scalar.activation(r1[:], pa1[:], ACTF.Relu, bias=G1B1[:])
                ph1 = psB.tile([1, 512], F32, tag="psC", name="ph1")
                nc.tensor.matmul(out=ph1[:], lhsT=G1W2[:], rhs=r1[:], start=True, stop=True)
                nc.scalar.activation(r0[0:1, sl], ph1[:], ACTF.Exp, bias=G1B2[:])
                pa2 = psB.tile([HID, 512], F32, tag="psC")
                nc.tensor.matmul(out=pa2[:], lhsT=G2W1[:], rhs=feat[:, sl],
                                 start=True, stop=True)
                r2 = yp.tile([HID, 512], F32, tag="y", name="r2", bufs=3)
                nc.scalar.activation(r2[:], pa2[:], ACTF.Relu, bias=G2B1[:])
                ph2 = psB.tile([DIM, 512], F32, tag="psC")
                nc.tensor.matmul(out=ph2[:], lhsT=G2W2[:], rhs=r2[:], start=True, stop=True)
                nc.vector.tensor_scalar(out=h2T[:, sl], in0=ph2[:], scalar1=G2B2[:],
                                        scalar2=None, op0=OP.add)

            
            hg = c1.tile([DIM, 8], F32, tag="hg")
            for j in range(8):
                lo, hi = GBOUND[j], GBOUND[j + 1]
                cnt = hi - lo
                sg = sm.tile([1, 1], F32, tag="sg")
                nc.vector.tensor_reduce(out=sg[:], in_=r0[0:1, lo:hi],
                                        axis=mybir.AxisListType.X, op=OP.add)
                nc.vector.tensor_scalar_mul(sg[:], sg[:], float(cnt))
                rg = sm.tile([1, 1], F32, tag="rg")
                nc.vector.reciprocal(out=rg[:], in_=sg[:])
                nc.vector.tensor_scalar(out=r0[0:1, lo:hi], in0=r0[0:1, lo:hi],
                                        scalar1=rg[:], scalar2=None, op0=OP.mult)
                pw = psB.tile([P, 512], F32, tag="psC")
                nc.tensor.matmul(out=pw[0:P, 0:cnt], lhsT=ones1_128[:],
                                 rhs=r0[0:1, lo:hi], start=True, stop=True)
                wh = yp.tile([DIM, 512], F32, tag="y", name="wh", bufs=3)
                nc.vector.tensor_tensor(out=wh[:, 0:cnt], in0=h2T[:, lo:hi],
                                        in1=pw[0:DIM, 0:cnt], op=OP.mult)
                nc.vector.tensor_reduce(out=hg[:, j:j + 1], in_=wh[:, 0:cnt],
                                        axis=mybir.AxisListType.X, op=OP.add)

            pp1 = psB.tile([HID, 8], F32, tag="psC")
            nc.tensor.matmul(out=pp1[:], lhsT=PW1[:], rhs=hg[:], start=True, stop=True)
            rp1 = sm.tile([HID, 8], F32, tag="rp1")
            nc.scalar.activation(rp1[:], pp1[:], ACTF.Relu, bias=PB1[:])
            pp2 = psB.tile([HID, 8], F32, tag="psC")
            nc.tensor.matmul(out=pp2[:], lhsT=PW2[:], rhs=rp1[:], start=True, stop=True)
            rp2 = sm.tile([HID, 8], F32, tag="rp2")
            nc.scalar.activation(rp2[:], pp2[:], ACTF.Relu, bias=PB2[:])
            pp3 = psB.tile([1, 8], F32, tag="psC")
            nc.tensor.matmul(out=pp3[:], lhsT=PW3[:], rhs=rp2[:], start=True, stop=True)
            ores = sm.tile([1, 8], F32, tag="ores")
            nc.vector.tensor_scalar(out=ores[:], in0=pp3[:], scalar1=PB3[:],
                                    scalar2=None, op0=OP.add)
            nc.sync.dma_start(out=eout[:], in_=ores[:])
            ps_stackC.__exit__(None, None, None)

    lower_extended_insts(nc)
    return nc


def _prep_host(inputs):
    src = np.asarray(inputs['edge_index'][0]).astype(np.int64)
    dst = np.asarray(inputs['edge_index'][1]).astype(np.int64)
    attr = np.asarray(inputs['edge_attr_idx']).astype(np.int64)
    x_idx = np.asarray(inputs['x_idx']).astype(np.int64)
    emb = np.asarray(inputs['emb']).astype(np.float32)

    conv_We = np.asarray(inputs['conv_We'], np.float32)
    att_e = np.asarray(inputs['conv_att_edge'], np.float32)
    V = np.stack([conv_We[l] @ att_e[l] for l in range(L)], 1)    # [128, 6]
    t_all = (emb @ V).astype(np.float32)                          # [128, 6]

    owner = dst // NPC
    srcg = ((src // NPC) * NPAD + src % NPC).astype(np.int64)     # padded global id

    per_core = []
    for c in range(NC):
        m = np.where(owner == c)[0]
        dl = (dst[m] - c * NPC).astype(np.int64)
        order = np.argsort(dl, kind='stable')
        eidx = m[order]
        dl = dl[order]
        # segment starts per node
        counts = np.bincount(dl, minlength=NPC)
        starts = np.zeros(NPC + 1, np.int64)
        np.cumsum(counts, out=starts[1:])
        per_core.append((eidx, dl, counts, starts))

    # choose M: smallest with per-group spill <= 128
    M = 8
    while True:
        ok = True
        for c in range(NC):
            counts = np.zeros(NPAD, np.int64)
            counts[:NPC] = per_core[c][2]
            sp = np.maximum(counts - M, 0).reshape(GRP, P).sum(1)
            if sp.max() > P:
                ok = False
                break
        if ok:
            break
        M += 1

    CG = M + 2
    C = GRP * CG
    cores = []
    for c in range(NC):
        eidx, dl, counts, starts = per_core[c]
        idxflat = np.zeros(C * P, np.int64)
        ae = np.zeros((P, C, 7), np.float32)
        sdstl = np.zeros((P, GRP), np.float32)
        ael = np.zeros((NPC,), np.float32)
        for g in range(GRP):
            base = g * CG
            sp_src, sp_dst, sp_attr, sp_lane = [], [], [], []
            for p in range(P):
                n = g * P + p
                if n >= NPC:
                    continue
                s0, cnt = starts[n], counts[n]
                take = min(cnt, M)
                es = eidx[s0:s0 + cnt]
                for k in range(take):
                    ch = base + k
                    idxflat[ch * P + p] = srcg[es[k]]
                    ae[p, ch, 0:6] = t_all[attr[es[k]]]
                    ae[p, ch, 6] = 1.0
                if cnt > M:
                    for k in range(M, cnt):
                        sp_src.append(srcg[es[k]])
                        sp_dst.append(c * NPAD + n)
                        sp_attr.append(attr[es[k]])
                        sp_lane.append(p)
            ns = len(sp_src)
            assert ns <= P
            chs, chd = base + M, base + M + 1
            for j in range(ns):
                idxflat[chs * P + j] = sp_src[j]
                idxflat[chd * P + j] = sp_dst[j]
                ae[j, chs, 0:6] = t_all[sp_attr[j]]
                ae[j, chs, 6] = 1.0
                sdstl[j, g] = float(sp_lane[j])
            # trailing empty slots of the last (spill-dst) chunk: mark -1 so
            # the gather ucode trims them (descriptors skipped; stale Gb data
            # in those lanes is masked by ae[...,6]=0)
            idxflat[chd * P + ns:(chd + 1) * P] = -1
        # wrapped int16 index layout, replicated per 16-partition group
        NIDX = C * P
        idxw = np.zeros((P, NIDX // 16), np.int16)
        fl = idxflat.astype(np.int16)
        for r in range(16):
            idxw[r::16, :] = fl[r::16].reshape(1, -1)
        # per-node loop attr (host: pure index/weight math)
        ae_sum = np.zeros((NPC, L), np.float32)
        deg = counts.astype(np.float32)
        np.add.at(ae_sum, dl, t_all[attr[eidx]])
        ael = ae_sum / np.maximum(deg, 1.0)[:, None]
        ael_pad = np.zeros((NPAD, L), np.float32)
        ael_pad[:NPC] = ael
        aeloop = ael_pad.reshape(GRP, P, L).transpose(1, 0, 2).copy()
        cores.append(dict(e_idxw=idxw, e_ae=ae, e_sdstl=sdstl, e_aeloop=aeloop))

    # ---- shared weights
    conv_W = np.asarray(inputs['conv_W'], np.float32)
    att_s = np.asarray(inputs['conv_att_src'], np.float32)
    att_d = np.asarray(inputs['conv_att_dst'], np.float32)
    conv_b = np.asarray(inputs['conv_b'], np.float32)
    m1 = np.asarray(inputs['mlp_W1'], np.float32)
    m2 = np.asarray(inputs['mlp_W2'], np.float32)
    m3 = np.asarray(inputs['mlp_W3'], np.float32)
    b1 = np.asarray(inputs['mlp_b1'], np.float32)
    b2 = np.asarray(inputs['mlp_b2'], np.float32)
    b3 = np.asarray(inputs['mlp_b3'], np.float32)
    b1_eff = np.stack([conv_b[l] @ m1[l] + b1[l] for l in range(L)], 1)

    shared = dict(
        w_iota=np.broadcast_to(np.arange(P, dtype=np.float32)[None, :], (P, P)).copy(),
        w_iotac=np.arange(P, dtype=np.float32).reshape(P, 1),
        w_ident=np.eye(P, dtype=np.float32),
        w_emb=emb,
        w_conv=np.concatenate([conv_W[l] for l in range(L)], 1),
        w_att=np.concatenate([np.stack([att_s[l], att_d[l]], 1) for l in range(L)], 1),
        w_m1=np.concatenate([m1[l] for l in range(L)], 1),
        w_m2=np.concatenate([m2[l] for l in range(L)], 1),
        w_m3=np.concatenate([m3[l] for l in range(L)], 1),
        w_b1=b1_eff,
        w_b2=b2.T.copy(),
        w_b3=b3.T.copy(),
        w_eps=np.broadcast_to((1.0 - np.arange(L, dtype=np.float32) * 1e-7)[None, :],
                              (P, L)).copy(),
        w_g1w1=np.asarray(inputs['g1_W1'], np.float32),
        w_g1b1=np.asarray(inputs['g1_b1'], np.float32).reshape(HID, 1),
        w_g1w2=np.asarray(inputs['g1_W2'], np.float32),
        w_g1b2=np.asarray(inputs['g1_b2'], np.float32).reshape(1, 1),
        w_g2w1=np.asarray(inputs['g2_W1'], np.float32),
        w_g2b1=np.asarray(inputs['g2_b1'], np.float32).reshape(HID, 1),
        w_g2w2=np.asarray(inputs['g2_W2'], np.float32),
        w_g2b2=np.asarray(inputs['g2_b2'], np.float32).reshape(DIM, 1),
        w_pw1=np.asarray(inputs['p_W1'], np.float32),
        w_pb1=np.asarray(inputs['p_b1'], np.float32).reshape(HID, 1),
        w_pw2=np.asarray(inputs['p_W2'], np.float32),
        w_pb2=np.asarray(inputs['p_b2'], np.float32).reshape(HID, 1),
        w_pw3=np.asarray(inputs['p_W3'], np.float32),
        w_pb3=np.asarray(inputs['p_b3'], np.float32).reshape(1, 1),
    )

    in_maps = []
    for c in range(NC):
        xi = np.full(NPAD, -1.0, np.float32)
        xi[:NPC] = x_idx[c * NPC:(c + 1) * NPC].astype(np.float32)
        mm = dict(shared)
        mm.update(cores[c])
        mm['e_xidx'] = xi
        in_maps.append(mm)
    return M, in_maps


def kernel(**inputs):
    M, in_maps = _prep_host(inputs)
    if M not in _cache:
        _cache[M] = _build(M)
    nc = _cache[M]
    res = run_bass_kernel_spmd(nc, in_maps, core_ids=list(range(NC)))
    out = np.concatenate([np.asarray(res.results[c]['out']).reshape(8)
                          for c in range(NC)])
    return out.astype(np.float32)


if __name__ == "__main__":
    import jax
    sys.path.insert(0, '/root/problem')
    import reference as R
    with jax.default_device(jax.devices('cpu')[0]):
        inp = R.setup_inputs()
        exp = np.asarray(R.reference(**inp))
    inp = {k: np.asarray(v) for k, v in inp.items()}
    act = kernel(**inp)
    rel = np.linalg.norm(act - exp) / np.linalg.norm(exp)
    print("Relative error:", rel)



# revision 24
# speedup vs baseline: 1.1799x; 1.0313x over previous
"""Trainium2 Bass kernel for nn_GAT_34059090657327 (6-layer GAT + JKN + attention pooling).

Distribution (8 NeuronCores, SPMD):
  - Nodes dst-sharded: core c owns nodes [2500c, 2500(c+1)), padded to 2560 (20 groups of 128).
  - Edges live on the core owning their dst. Edge slots are dst-major: lane p of group g
    holds up to M in-edges of node g*128+p (along the free dim), overflow edges go to one
    spill chunk per group. With this layout the weighted scatter-add is a PSUM-accumulated
    matmul with an *identity* stationary operand (plus one one-hot matmul for the spill
    chunk), a_d broadcasts per-partition, and the aggregation lands node-major so the
    softmax division is a per-partition scalar op.
  - Per layer: h|a_s|a_d computed for owned nodes (feature-major matmuls), AllGathered into
    a replicated bf16 [20480, 128] DRAM table (row = [h(64)|a_s|a_d|pad]); per-edge rows
    fetched by src via gpsimd dma_gather (256B bf16 descriptors; trailing empty spill-dst
    slots are -1 so the ucode trims their descriptors); attention exp/normalize on-chip in
    f32 (a_s columns cast out of the bf16 gather buffer first).
  - JKN argmax is folded into each layer's MLP slab loop (running max of eps-weighted
    squared norms + masked feature update), removing the serial post-loop JKN pass.
  - Host-side prep is index/weight-only: edge partitioning + packing, the folded edge-
    attention table t = emb @ conv_We @ att_edge gathered per-edge, its per-node mean
    (self-loop attr), and conv bias folded into mlp b1. All x/h-dependent math is on-device.
  - Softmax max-subtraction dropped (logits are O(0.3) for this model; validated exact).
  - JKN argmax via eps-perturbed squared norms; per-graph pooling is core-local (graph
    boundaries align with the node sharding); final MLP -> [8] per core, host concat.
"""
import numpy as np
import sys

sys.path.insert(0, '/opt/trn_rl_repo')

import concourse.bass as bass
import concourse.mybir as mybir
import concourse.tile as tile
from concourse import library_config
from concourse.bass import AP
from concourse.bass_utils import run_bass_kernel_spmd
from concourse.library_overlay import lower_extended_insts
from concourse.tile_rust import add_dep_helper

F32 = mybir.dt.float32
BF16 = mybir.dt.bfloat16
I16 = mybir.dt.int16
OP = mybir.AluOpType
ACTF = mybir.ActivationFunctionType

N, E, NG, DIM, HID, L = 20000, 320000, 64, 128, 64, 6
NC = 8
NPC = N // NC            # 2500
P = 128
GRP = 20                 # node groups of 128 per core
NPAD = GRP * P           # 2560
LRELU = 0.2
GBOUND = [int(np.ceil(j * NPC / 8)) for j in range(9)]  # local graph boundaries

_cache = {}

# ---------------------------------------------------------------------------
# This walrus build encodes only ONE semaphore wait/update per TPB_CTRL
# instruction ("Too many sync wait commands" on the Tile tail drain). Split
# extra waits onto preceding NoOps at BIR-serialization time.
import json as _json


def _fix_prep_sems(j: dict) -> None:
    """Point each SWDGE prep's DMA-completion sem at the Tile DMASW lane sem
    its consumers wait on. Tile assigns gen_mode=1 preps round-robin to the 8
    DMASW lanes (pass 1) and emits consumer waits against those lane sems, but
    leaves the user-passed `sem=` on the prep — so nothing ever increments the
    lane sems. Rewrite on_update[0] of the k-th prep (program order) to lane
    k%8's sem."""
    lanes = {}
    import re
    for fn in j["functions"]:
        for bb in fn["blocks"]:
            for inst in bb["instructions"]:
                si = inst.get("sync_info") or {}
                for w in (si.get("on_wait") or []) + (si.get("on_update") or []):
                    m = re.match(r"DMASW(\d+)_", w.get("ant_name", ""))
                    if m:
                        lanes[int(m.group(1))] = (w["ant_name"], w["id"])
    if not lanes:
        return
    nl = max(lanes) + 1
    assert sorted(lanes) == list(range(nl)), lanes
    k = 0
    for fn in j["functions"]:
        for bb in fn["blocks"]:
            for inst in bb["instructions"]:
                if inst["opcode"] != "DMAGatherAnt":
                    continue
                ups = (inst.get("sync_info") or {}).get("on_update") or []
                if ups and ups[0].get("ant_name") == "gsem":
                    name, sid = lanes[k % nl]
                    ups[0]["ant_name"] = name
                    ups[0]["id"] = sid
                    k += 1


def _split_multiwaits(js: bytes) -> bytes:
    j = _json.loads(js)
    _fix_prep_sems(j)
    n = 0
    for fn in j["functions"]:
        for bb in fn["blocks"]:
            out = []
            for inst in bb["instructions"]:
                si = inst.get("sync_info") or {}
                waits = si.get("on_wait") or []
                if len(waits) > 1:
                    for w in waits[:-1]:
                        n += 1
                        out.append({
                            "name": inst["name"] + f"_w{n}", "opcode": "NoOp",
                            "engine": inst["engine"], "ins": [], "outs": [],
                            "sync_info": {"on_wait": [w], "on_update": []},
                        })
                    si["on_wait"] = [waits[-1]]
                out.append(inst)
                ups = si.get("on_update") or []
                if len(ups) > 1 and inst["opcode"] in ("NoOp", "Drain", "EventSemaphore"):
                    si["on_update"] = [ups[0]]
                    for u in ups[1:]:
                        n += 1
                        out.append({
                            "name": inst["name"] + f"_u{n}", "opcode": "NoOp",
                            "engine": inst["engine"], "ins": [], "outs": [],
                            "sync_info": {"on_wait": [], "on_update": [u]},
                        })
            bb["instructions"] = out
    return _json.dumps(j).encode()


if not getattr(bass.Bass, "_mw_patched", False):
    _orig_to_json_bytes = bass.Bass.to_json_bytes

    def _to_json_bytes_patched(self, *a, **k):
        return _split_multiwaits(_orig_to_json_bytes(self, *a, **k))

    bass.Bass.to_json_bytes = _to_json_bytes_patched
    bass.Bass._mw_patched = True


def _bc(ap, pos, count):
    """Insert a stride-0 (broadcast) dim of `count` at free-dim position `pos`."""
    lst = [list(x) for x in ap.ap]
    lst.insert(1 + pos, [0, count])
    return AP(ap.tensor, ap.offset, lst)


def _build(M):
    CG = M + 2               # chunks per group: M main + spill + spill-dst
    C = GRP * CG             # total chunks per core
    NIG = CG * P             # gather indices per group

    nc = bass.Bass(num_devices=NC)

    # ---------------- inputs ----------------
    e_idxw = nc.dram_tensor("e_idxw", [P, C * 8], I16, kind="ExternalInput")
    e_ae = nc.dram_tensor("e_ae", [P, C, 7], F32, kind="ExternalInput")
    e_sdstl = nc.dram_tensor("e_sdstl", [P, GRP], F32, kind="ExternalInput")
    e_aeloop = nc.dram_tensor("e_aeloop", [P, GRP, L], F32, kind="ExternalInput")
    e_xidx = nc.dram_tensor("e_xidx", [NPAD], F32, kind="ExternalInput")
    w_iota = nc.dram_tensor("w_iota", [P, P], F32, kind="ExternalInput")
    w_iotac = nc.dram_tensor("w_iotac", [P, 1], F32, kind="ExternalInput")
    w_ident = nc.dram_tensor("w_ident", [P, P], F32, kind="ExternalInput")
    w_emb = nc.dram_tensor("w_emb", [P, P], F32, kind="ExternalInput")
    w_conv = nc.dram_tensor("w_conv", [P, L * HID], F32, kind="ExternalInput")
    w_att = nc.dram_tensor("w_att", [P, L * 66], F32, kind="ExternalInput")
    w_m1 = nc.dram_tensor("w_m1", [HID, L * HID], F32, kind="ExternalInput")
    w_m2 = nc.dram_tensor("w_m2", [HID, L * HID], F32, kind="ExternalInput")
    w_m3 = nc.dram_tensor("w_m3", [HID, L * DIM], F32, kind="ExternalInput")
    w_b1 = nc.dram_tensor("w_b1", [HID, L], F32, kind="ExternalInput")
    w_b2 = nc.dram_tensor("w_b2", [HID, L], F32, kind="ExternalInput")
    w_b3 = nc.dram_tensor("w_b3", [DIM, L], F32, kind="ExternalInput")
    w_eps = nc.dram_tensor("w_eps", [P, L], F32, kind="ExternalInput")
    w_g1w1 = nc.dram_tensor("w_g1w1", [DIM, HID], F32, kind="ExternalInput")
    w_g1b1 = nc.dram_tensor("w_g1b1", [HID, 1], F32, kind="ExternalInput")
    w_g1w2 = nc.dram_tensor("w_g1w2", [HID, 1], F32, kind="ExternalInput")
    w_g1b2 = nc.dram_tensor("w_g1b2", [1, 1], F32, kind="ExternalInput")
    w_g2w1 = nc.dram_tensor("w_g2w1", [DIM, HID], F32, kind="ExternalInput")
    w_g2b1 = nc.dram_tensor("w_g2b1", [HID, 1], F32, kind="ExternalInput")
    w_g2w2 = nc.dram_tensor("w_g2w2", [HID, DIM], F32, kind="ExternalInput")
    w_g2b2 = nc.dram_tensor("w_g2b2", [DIM, 1], F32, kind="ExternalInput")
    w_pw1 = nc.dram_tensor("w_pw1", [DIM, HID], F32, kind="ExternalInput")
    w_pb1 = nc.dram_tensor("w_pb1", [HID, 1], F32, kind="ExternalInput")
    w_pw2 = nc.dram_tensor("w_pw2", [HID, HID], F32, kind="ExternalInput")
    w_pb2 = nc.dram_tensor("w_pb2", [HID, 1], F32, kind="ExternalInput")
    w_pw3 = nc.dram_tensor("w_pw3", [HID, 1], F32, kind="ExternalInput")
    w_pb3 = nc.dram_tensor("w_pb3", [1, 1], F32, kind="ExternalInput")
    eout = nc.dram_tensor("out", [1, 8], F32, kind="ExternalOutput")

    with tile.TileContext(nc) as tc:
        with tc.tile_pool(name="c1", bufs=1) as c1, \
             tc.tile_pool(name="big", bufs=2) as bigp, \
             tc.tile_pool(name="gp", bufs=2) as gp, \
             tc.tile_pool(name="zp", bufs=2) as zp, \
             tc.tile_pool(name="sm", bufs=3) as sm, \
             tc.tile_pool(name="stg", bufs=2) as stgp, \
             tc.tile_pool(name="yp", bufs=2) as yp, \
             tc.tile_pool(name="dr", bufs=1, space="DRAM") as dr:
            ps_stack = tc.tile_pool(name="psA", bufs=4, space="PSUM")
            psA = ps_stack.__enter__()
            ps_stackB = tc.tile_pool(name="psB", bufs=3, space="PSUM")
            psB = ps_stackB.__enter__()

            rel = nc.gpsimd.load_library(library_config.mlp)
            nig_reg = nc.gpsimd.to_reg(CG * P)

            def load(t, shape, tag, dtype=F32):
                s = c1.tile(shape, dtype, tag=tag)
                nc.sync.dma_start(out=s[:], in_=t[:])
                return s

            ident = load(w_ident, [P, P], "ident")
            iota_f = load(w_iota, [P, P], "iota_f")
            iotac = load(w_iotac, [P, 1], "iotac")
            Wconv = load(w_conv, [P, L * HID], "Wconv")
            Watt = load(w_att, [P, L * 66], "Watt")
            Wm1 = load(w_m1, [HID, L * HID], "Wm1")
            Wm2 = load(w_m2, [HID, L * HID], "Wm2")
            Wm3 = load(w_m3, [HID, L * DIM], "Wm3")
            B1 = load(w_b1, [HID, L], "B1")
            B2 = load(w_b2, [HID, L], "B2")
            B3 = load(w_b3, [DIM, L], "B3")
            Emb = load(w_emb, [P, P], "Emb")
            Eps = load(w_eps, [P, L], "Eps")
            G1W1 = load(w_g1w1, [DIM, HID], "G1W1")
            G1B1 = load(w_g1b1, [HID, 1], "G1B1")
            G1W2 = load(w_g1w2, [HID, 1], "G1W2")
            G1B2 = load(w_g1b2, [1, 1], "G1B2")
            G2W1 = load(w_g2w1, [DIM, HID], "G2W1")
            G2B1 = load(w_g2b1, [HID, 1], "G2B1")
            G2W2 = load(w_g2w2, [HID, DIM], "G2W2")
            G2B2 = load(w_g2b2, [DIM, 1], "G2B2")
            PW1 = load(w_pw1, [DIM, HID], "PW1")
            PB1 = load(w_pb1, [HID, 1], "PB1")
            PW2 = load(w_pw2, [HID, HID], "PW2")
            PB2 = load(w_pb2, [HID, 1], "PB2")
            PW3 = load(w_pw3, [HID, 1], "PW3")
            PB3 = load(w_pb3, [1, 1], "PB3")

            idxw = load(e_idxw, [P, C * 8], "idxw", dtype=I16)
            AE = load(e_ae, [P, C, 7], "AE")
            sdstl = load(e_sdstl, [P, GRP], "sdstl")
            aeloop = load(e_aeloop, [P, GRP, L], "aeloop")

            ones1_128 = c1.tile([1, P], F32, tag="ones1_128")
            nc.vector.memset(ones1_128[:], 1.0)
            identb = c1.tile([P, P], BF16, tag="identb")
            nc.vector.tensor_copy(out=identb[:], in_=ident[:])
            asdf = c1.tile([P, GRP, 2], F32, tag="asdf")

            # x_idx broadcast to [128, NPAD] (partition-stride-0 DMA read)
            xidxb = bigp.tile([P, NPAD], F32, tag="xbig")
            nc.sync.dma_start(out=xidxb[:], in_=AP(e_xidx, 0, [[0, P], [1, NPAD]]))

            # x tiles (jkn entries) + initial x (feature-major [128 f, node])
            xs = [c1.tile([P, NPAD], F32, tag=f"xs{l}", name=f"xs{l}") for l in range(L)]
            x_init = bigp.tile([P, NPAD], F32, tag="xbig")
            for s in range(5):
                sl = slice(s * 512, (s + 1) * 512)
                ohx = stgp.tile([P, 512], F32, tag="stg", name="ohx")
                nc.vector.tensor_scalar(out=ohx[:], in0=xidxb[:, sl],
                                        scalar1=iotac[:], scalar2=None,
                                        op0=OP.is_equal)
                px = psB.tile([P, 512], F32, tag="psB")
                nc.tensor.matmul(out=px[:], lhsT=Emb[:], rhs=ohx[:], start=True, stop=True)
                nc.vector.tensor_copy(out=x_init[:, sl], in_=px[:])

            outc = c1.tile([HID, NPAD], F32, tag="outc")

            # DRAM comm buffers (Shared tensors allow a single writer -> one pair per layer)
            ag_ins = [dr.tile([NPAD, P], BF16, tag=f"ag_in{l}", name=f"ag_in{l}")
                      for l in range(L)]
            ag_outs = [dr.tile([NC * NPAD, P], BF16, tag=f"ag_out{l}", name=f"ag_out{l}",
                               addr_space="Shared") for l in range(L)]

            feat = bigp.tile([P, NPAD], F32, tag="xbig", name="feat", bufs=2)
            mx = c1.tile([1, NPAD], F32, tag="mx")
            r0 = c1.tile([1, NPAD], F32, tag="r0")

            # =================== layers ===================
            for l in range(L):
                x_cur = x_init if l == 0 else xs[l - 1]

                # ---- h | a_s | a_d for owned nodes; node-major staging -> ag_in
                nm = stgp.tile([P, GRP, P], BF16, tag="nm", bufs=1)
                nc.vector.memset(nm[:, :, 66:P], 0.0)
                # node-major h|a_s|a_d directly: per 128-node chunk,
                # out[n, :] = x_chunk^T @ [Wconv | Wconv@att] (att folded on host)
                for g in range(GRP):
                    cs = slice(g * P, (g + 1) * P)
                    ptr = psA.tile([P, 66], F32, tag="psA")
                    nc.tensor.matmul(out=ptr[:], lhsT=x_cur[:, cs],
                                     rhs=Watt[:, l * 66:(l + 1) * 66],
                                     start=True, stop=True)
                    nc.vector.tensor_copy(out=nm[:, g, 0:66], in_=ptr[:])
                    nc.vector.tensor_copy(out=asdf[:, g, :], in_=ptr[:, 64:66])
                nc.sync.dma_start(out=ag_ins[l][:].rearrange("(g p) c -> p g c", p=P),
                                  in_=nm[:])

                # ---- AllGather the node table
                nc.gpsimd.collective_compute(
                    "AllGather", OP.bypass, replica_groups=[list(range(NC))],
                    ins=[ag_ins[l][:]], outs=[ag_outs[l][:]])

                # ---- self-loop weights, node-major [128, GRP]
                wloop = sm.tile([P, GRP], F32, tag="wloop")
                zt = sm.tile([P, GRP], F32, tag="zt")
                nc.vector.tensor_tensor(out=zt[:], in0=asdf[:, :, 0], in1=asdf[:, :, 1],
                                        op=OP.add)
                nc.vector.tensor_tensor(out=zt[:], in0=zt[:], in1=aeloop[:, :, l],
                                        op=OP.add)
                t2 = sm.tile([P, GRP], F32, tag="zt2")
                nc.vector.tensor_scalar_mul(t2[:], zt[:], LRELU)
                nc.vector.tensor_tensor(out=zt[:], in0=zt[:], in1=t2[:], op=OP.max)
                nc.scalar.activation(wloop[:], zt[:], ACTF.Exp)

                # ---- per-group edge processing
                for g in range(GRP):
                    gs = g * CG
                    Gb = gp.tile([P, CG, P], BF16, tag="Gb", bufs=3)
                    gi = nc.gpsimd.dma_gather(
                        out_ap=Gb[:], in_ap=ag_outs[l][:],
                        idxs_ap=idxw[:, gs * 8:(gs + CG) * 8],
                        num_idxs=NIG, num_idxs_reg=nig_reg, elem_size=P,
                        single_packet=False)
                    add_dep_helper(gi.ins, rel.ins, False, "needs mlp lib")
                    # logits -> w  (main slots 0..M-1, spill slot M)
                    zcp = zp.tile([P, M + 2], F32, tag="zcp")
                    nc.vector.tensor_copy(out=zcp[:, 0:M + 1], in_=Gb[:, 0:M + 1, 64])
                    nc.vector.tensor_copy(out=zcp[:, M + 1:M + 2],
                                          in_=Gb[:, M + 1, 65:66])
                    z = zp.tile([P, M + 1], F32, tag="z")
                    nc.vector.tensor_scalar(out=z[:, 0:M], in0=zcp[:, 0:M],
                                            scalar1=asdf[:, g, 1:2], scalar2=None,
                                            op0=OP.add)
                    nc.vector.tensor_tensor(out=z[:, 0:M], in0=z[:, 0:M],
                                            in1=AE[:, gs:gs + M, l], op=OP.add)
                    nc.vector.tensor_tensor(out=z[:, M:M + 1], in0=zcp[:, M:M + 1],
                                            in1=zcp[:, M + 1:M + 2], op=OP.add)
                    nc.vector.tensor_tensor(out=z[:, M:M + 1], in0=z[:, M:M + 1],
                                            in1=AE[:, gs + M, l:l + 1], op=OP.add)
                    t0 = zp.tile([P, M + 1], F32, tag="t0")
                    nc.vector.tensor_scalar_mul(t0[:], z[:], LRELU)
                    nc.vector.tensor_tensor(out=z[:], in0=z[:], in1=t0[:], op=OP.max)
                    w = zp.tile([P, M + 1], F32, tag="w")
                    nc.scalar.activation(w[:], z[:], ACTF.Exp)
                    nc.vector.tensor_tensor(out=w[:], in0=w[:], in1=AE[:, gs:gs + M + 1, 6],
                                            op=OP.mult)
                    wb = zp.tile([P, M + 1], BF16, tag="wb")
                    nc.vector.tensor_copy(out=wb[:], in_=w[:])
                    # denom column + scale rows by w
                    nc.vector.memset(Gb[:, 0:M + 1, 64:65], 1.0)
                    nc.vector.tensor_tensor(out=Gb[:, 0:M + 1, 0:65],
                                            in0=Gb[:, 0:M + 1, 0:65],
                                            in1=_bc(wb[:], 1, 65), op=OP.mult)
                    # self-loop message
                    smsg = sm.tile([P, 65], BF16, tag="smsg")
                    nc.vector.tensor_scalar(out=smsg[:, 0:64], in0=nm[:, g, 0:64],
                                            scalar1=wloop[:, g:g + 1], scalar2=None,
                                            op0=OP.mult)
                    nc.vector.tensor_copy(out=smsg[:, 64:65], in_=wloop[:, g:g + 1])
                    # spill one-hot
                    oh = sm.tile([P, P], BF16, tag="oh")
                    nc.vector.tensor_scalar(out=oh[:], in0=iota_f[:],
                                            scalar1=sdstl[:, g:g + 1], scalar2=None,
                                            op0=OP.is_equal)
                    # scatter-accumulate (node-major)
                    pg = psA.tile([P, 65], F32, tag="psA")
                    for k in range(M):
                        nc.tensor.matmul(out=pg[:], lhsT=identb[:], rhs=Gb[:, k, 0:65],
                                         start=(k == 0), stop=False)
                    nc.tensor.matmul(out=pg[:], lhsT=identb[:], rhs=smsg[:],
                                     start=False, stop=False)
                    nc.tensor.matmul(out=pg[:], lhsT=oh[:], rhs=Gb[:, M, 0:65],
                                     start=False, stop=True)
                    # normalize + transpose to feature-major
                    rec = sm.tile([P, 1], F32, tag="rec")
                    nc.vector.reciprocal(out=rec[:], in_=pg[:, 64:65])
                    onm = sm.tile([P, 64], F32, tag="onm")
                    nc.vector.tensor_scalar(out=onm[:], in0=pg[:, 0:64], scalar1=rec[:],
                                            scalar2=None, op0=OP.mult)
                    ptr2 = psA.tile([64, P], F32, tag="psA")
                    nc.tensor.transpose(out=ptr2[:], in_=onm[:], identity=ident[:])
                    nc.vector.tensor_copy(out=outc[:, g * P:(g + 1) * P], in_=ptr2[:])

                # ---- MLP (feature-major)
                for s in range(5):
                    sl = slice(s * 512, (s + 1) * 512)
                    p1 = psB.tile([HID, 512], F32, tag="psB")
                    nc.tensor.matmul(out=p1[:], lhsT=Wm1[:, l * HID:(l + 1) * HID],
                                     rhs=outc[:, sl], start=True, stop=True)
                    y1 = yp.tile([HID, 512], F32, tag="y", name="y1", bufs=3)
                    nc.scalar.activation(y1[:], p1[:], ACTF.Relu, bias=B1[:, l:l + 1])
                    p2 = psB.tile([HID, 512], F32, tag="psB")
                    nc.tensor.matmul(out=p2[:], lhsT=Wm2[:, l * HID:(l + 1) * HID],
                                     rhs=y1[:], start=True, stop=True)
                    y2 = yp.tile([HID, 512], F32, tag="y", name="y2", bufs=3)
                    nc.scalar.activation(y2[:], p2[:], ACTF.Relu, bias=B2[:, l:l + 1])
                    p3 = psB.tile([P, 512], F32, tag="psB")
                    nc.tensor.matmul(out=p3[:], lhsT=Wm3[:, l * DIM:(l + 1) * DIM],
                                     rhs=y2[:], start=True, stop=True)
                    nc.vector.tensor_scalar(out=xs[l][:, sl], in0=p3[:],
                                            scalar1=B3[:, l:l + 1], scalar2=None,
                                            op0=OP.add)
                    # incremental JKN: track running max eps-weighted sq-norm
                    # and the argmax layer's features
                    sq = sm.tile([P, 512], F32, tag="sq", name="sq", bufs=2)
                    nc.scalar.activation(sq[:], xs[l][:, sl], ACTF.Square)
                    pml = psB.tile([1, 512], F32, tag="psB", name="pml")
                    nc.tensor.matmul(out=pml[:], lhsT=Eps[:, l:l + 1], rhs=sq[:],
                                     start=True, stop=True)
                    if l == 0:
                        nc.vector.tensor_copy(out=mx[0:1, sl], in_=pml[:])
                        nc.vector.tensor_copy(out=feat[:, sl], in_=xs[0][:, sl])
                    else:
                        gt = sm.tile([1, 512], F32, tag="gt", name="gt", bufs=2)
                        nc.vector.tensor_tensor(out=gt[:], in0=pml[:],
                                                in1=mx[0:1, sl], op=OP.is_gt)
                        nc.vector.tensor_tensor(out=mx[0:1, sl], in0=mx[0:1, sl],
                                                in1=pml[:], op=OP.max)
                        pgt = psB.tile([P, 512], F32, tag="psB", name="pgt")
                        nc.tensor.matmul(out=pgt[:], lhsT=ones1_128[:], rhs=gt[:],
                                         start=True, stop=True)
                        df = sm.tile([P, 512], F32, tag="df", name="df", bufs=2)
                        nc.vector.tensor_tensor(out=df[:], in0=xs[l][:, sl],
                                                in1=feat[:, sl], op=OP.subtract)
                        nc.vector.tensor_tensor(out=df[:], in0=df[:], in1=pgt[:],
                                                op=OP.mult)
                        nc.vector.tensor_tensor(out=feat[:, sl], in0=feat[:, sl],
                                                in1=df[:], op=OP.add)

            # layer-phase PSUM pools -> pooling-phase pool
            ps_stackB.__exit__(None, None, None)
            ps_stack.__exit__(None, None, None)
            ps_stackC = tc.tile_pool(name="psC", bufs=2, space="PSUM")
            psC = ps_stackC.__enter__()
            psB = psC  # later phases allocate from psC

            # =================== pooling ===================
            h2T = c1.tile([DIM, NPAD], F32, tag="h2T")
            for s in range(5):
                sl = slice(s * 512, (s + 1) * 512)
                pa1 = psB.tile([HID, 512], F32, tag="psC")
                nc.tensor.matmul(out=pa1[:], lhsT=G1W1[:], rhs=feat[:, sl],
                                 start=True, stop=True)
                r1 = yp.tile([HID, 512], F32, tag="y", name="r1", bufs=3)
                nc.scalar.activation(r1[:], pa1[:], ACTF.Relu, bias=G1B1[:])
                ph1 = psB.tile([1, 512], F32, tag="psC", name="ph1")
                nc.tensor.matmul(out=ph1[:], lhsT=G1W2[:], rhs=r1[:], start=True, stop=True)
                nc.scalar.activation(r0[0:1, sl], ph1[:], ACTF.Exp, bias=G1B2[:])
                pa2 = psB.tile([HID, 512], F32, tag="psC")
                nc.tensor.matmul(out=pa2[:], lhsT=G2W1[:], rhs=feat[:, sl],
                                 start=True, stop=True)
                r2 = yp.tile([HID, 512], F32, tag="y", name="r2", bufs=3)
                nc.scalar.activation(r2[:], pa2[:], ACTF.Relu, bias=G2B1[:])
                ph2 = psB.tile([DIM, 512], F32, tag="psC")
                nc.tensor.matmul(out=ph2[:], lhsT=G2W2[:], rhs=r2[:], start=True, stop=True)
                nc.vector.tensor_scalar(out=h2T[:, sl], in0=ph2[:], scalar1=G2B2[:],
                                        scalar2=None, op0=OP.add)

            
            hg = c1.tile([DIM, 8], F32, tag="hg")
            for j in range(8):
                lo, hi = GBOUND[j], GBOUND[j + 1]
                cnt = hi - lo
                sg = sm.tile([1, 1], F32, tag="sg")
                nc.vector.tensor_reduce(out=sg[:], in_=r0[0:1, lo:hi],
                                        axis=mybir.AxisListType.X, op=OP.add)
                nc.vector.tensor_scalar_mul(sg[:], sg[:], float(cnt))
                rg = sm.tile([1, 1], F32, tag="rg")
                nc.vector.reciprocal(out=rg[:], in_=sg[:])
                nc.vector.tensor_scalar(out=r0[0:1, lo:hi], in0=r0[0:1, lo:hi],
                                        scalar1=rg[:], scalar2=None, op0=OP.mult)
                pw = psB.tile([P, 512], F32, tag="psC")
                nc.tensor.matmul(out=pw[0:P, 0:cnt], lhsT=ones1_128[:],
                                 rhs=r0[0:1, lo:hi], start=True, stop=True)
                wh = yp.tile([DIM, 512], F32, tag="y", name="wh", bufs=3)
                nc.vector.tensor_tensor(out=wh[:, 0:cnt], in0=h2T[:, lo:hi],
                                        in1=pw[0:DIM, 0:cnt], op=OP.mult)
                nc.vector.tensor_reduce(out=hg[:, j:j + 1], in_=wh[:, 0:cnt],
                                        axis=mybir.AxisListType.X, op=OP.add)

            pp1 = psB.tile([HID, 8], F32, tag="psC")
            nc.tensor.matmul(out=pp1[:], lhsT=PW1[:], rhs=hg[:], start=True, stop=True)
            rp1 = sm.tile([HID, 8], F32, tag="rp1")
            nc.scalar.activation(rp1[:], pp1[:], ACTF.Relu, bias=PB1[:])
            pp2 = psB.tile([HID, 8], F32, tag="psC")
            nc.tensor.matmul(out=pp2[:], lhsT=PW2[:], rhs=rp1[:], start=True, stop=True)
            rp2 = sm.tile([HID, 8], F32, tag="rp2")
            nc.scalar.activation(rp2[:], pp2[:], ACTF.Relu, bias=PB2[:])
            pp3 = psB.tile([1, 8], F32, tag="psC")
            nc.tensor.matmul(out=pp3[:], lhsT=PW3[:], rhs=rp2[:], start=True, stop=True)
            ores = sm.tile([1, 8], F32, tag="ores")
            nc.vector.tensor_scalar(out=ores[:], in0=pp3[:], scalar1=PB3[:],
                                    scalar2=None, op0=OP.add)
            nc.sync.dma_start(out=eout[:], in_=ores[:])
            ps_stackC.__exit__(None, None, None)

    lower_extended_insts(nc)
    return nc


def _prep_host(inputs):
    src = np.asarray(inputs['edge_index'][0]).astype(np.int64)
    dst = np.asarray(inputs['edge_index'][1]).astype(np.int64)
    attr = np.asarray(inputs['edge_attr_idx']).astype(np.int64)
    x_idx = np.asarray(inputs['x_idx']).astype(np.int64)
    emb = np.asarray(inputs['emb']).astype(np.float32)

    conv_We = np.asarray(inputs['conv_We'], np.float32)
    att_e = np.asarray(inputs['conv_att_edge'], np.float32)
    V = np.stack([conv_We[l] @ att_e[l] for l in range(L)], 1)    # [128, 6]
    t_all = (emb @ V).astype(np.float32)                          # [128, 6]

    owner = dst // NPC
    srcg = ((src // NPC) * NPAD + src % NPC).astype(np.int64)     # padded global id

    per_core = []
    for c in range(NC):
        m = np.where(owner == c)[0]
        dl = (dst[m] - c * NPC).astype(np.int64)
        order = np.argsort(dl, kind='stable')
        eidx = m[order]
        dl = dl[order]
        # segment starts per node
        counts = np.bincount(dl, minlength=NPC)
        starts = np.zeros(NPC + 1, np.int64)
        np.cumsum(counts, out=starts[1:])
        per_core.append((eidx, dl, counts, starts))

    # choose M: smallest with per-group spill <= 128
    M = 8
    while True:
        ok = True
        for c in range(NC):
            counts = np.zeros(NPAD, np.int64)
            counts[:NPC] = per_core[c][2]
            sp = np.maximum(counts - M, 0).reshape(GRP, P).sum(1)
            if sp.max() > P:
                ok = False
                break
        if ok:
            break
        M += 1

    CG = M + 2
    C = GRP * CG
    cores = []
    for c in range(NC):
        eidx, dl, counts, starts = per_core[c]
        idxflat = np.zeros(C * P, np.int64)
        ae = np.zeros((P, C, 7), np.float32)
        sdstl = np.zeros((P, GRP), np.float32)
        ael = np.zeros((NPC,), np.float32)
        for g in range(GRP):
            base = g * CG
            sp_src, sp_dst, sp_attr, sp_lane = [], [], [], []
            for p in range(P):
                n = g * P + p
                if n >= NPC:
                    continue
                s0, cnt = starts[n], counts[n]
                take = min(cnt, M)
                es = eidx[s0:s0 + cnt]
                for k in range(take):
                    ch = base + k
                    idxflat[ch * P + p] = srcg[es[k]]
                    ae[p, ch, 0:6] = t_all[attr[es[k]]]
                    ae[p, ch, 6] = 1.0
                if cnt > M:
                    for k in range(M, cnt):
                        sp_src.append(srcg[es[k]])
                        sp_dst.append(c * NPAD + n)
                        sp_attr.append(attr[es[k]])
                        sp_lane.append(p)
            ns = len(sp_src)
            assert ns <= P
            chs, chd = base + M, base + M + 1
            for j in range(ns):
                idxflat[chs * P + j] = sp_src[j]
                idxflat[chd * P + j] = sp_dst[j]
                ae[j, chs, 0:6] = t_all[sp_attr[j]]
                ae[j, chs, 6] = 1.0
                sdstl[j, g] = float(sp_lane[j])
            # trailing empty slots of the last (spill-dst) chunk: mark -1 so
            # the gather ucode trims them (descriptors skipped; stale Gb data
            # in those lanes is masked by ae[...,6]=0)
            idxflat[chd * P + ns:(chd + 1) * P] = -1
        # wrapped int16 index layout, replicated per 16-partition group
        NIDX = C * P
        idxw = np.zeros((P, NIDX // 16), np.int16)
        fl = idxflat.astype(np.int16)
        for r in range(16):
            idxw[r::16, :] = fl[r::16].reshape(1, -1)
        # per-node loop attr (host: pure index/weight math)
        ae_sum = np.zeros((NPC, L), np.float32)
        deg = counts.astype(np.float32)
        np.add.at(ae_sum, dl, t_all[attr[eidx]])
        ael = ae_sum / np.maximum(deg, 1.0)[:, None]
        ael_pad = np.zeros((NPAD, L), np.float32)
        ael_pad[:NPC] = ael
        aeloop = ael_pad.reshape(GRP, P, L).transpose(1, 0, 2).copy()
        cores.append(dict(e_idxw=idxw, e_ae=ae, e_sdstl=sdstl, e_aeloop=aeloop))

    # ---- shared weights
    conv_W = np.asarray(inputs['conv_W'], np.float32)
    att_s = np.asarray(inputs['conv_att_src'], np.float32)
    att_d = np.asarray(inputs['conv_att_dst'], np.float32)
    conv_b = np.asarray(inputs['conv_b'], np.float32)
    m1 = np.asarray(inputs['mlp_W1'], np.float32)
    m2 = np.asarray(inputs['mlp_W2'], np.float32)
    m3 = np.asarray(inputs['mlp_W3'], np.float32)
    b1 = np.asarray(inputs['mlp_b1'], np.float32)
    b2 = np.asarray(inputs['mlp_b2'], np.float32)
    b3 = np.asarray(inputs['mlp_b3'], np.float32)
    b1_eff = np.stack([conv_b[l] @ m1[l] + b1[l] for l in range(L)], 1)

    shared = dict(
        w_iota=np.broadcast_to(np.arange(P, dtype=np.float32)[None, :], (P, P)).copy(),
        w_iotac=np.arange(P, dtype=np.float32).reshape(P, 1),
        w_ident=np.eye(P, dtype=np.float32),
        w_emb=emb,
        w_conv=np.concatenate([conv_W[l] for l in range(L)], 1),
        w_att=np.concatenate(
            [np.concatenate([conv_W[l],
                             conv_W[l] @ np.stack([att_s[l], att_d[l]], 1)], 1)
             for l in range(L)], 1),
        w_m1=np.concatenate([m1[l] for l in range(L)], 1),
        w_m2=np.concatenate([m2[l] for l in range(L)], 1),
        w_m3=np.concatenate([m3[l] for l in range(L)], 1),
        w_b1=b1_eff,
        w_b2=b2.T.copy(),
        w_b3=b3.T.copy(),
        w_eps=np.broadcast_to((1.0 - np.arange(L, dtype=np.float32) * 1e-7)[None, :],
                              (P, L)).copy(),
        w_g1w1=np.asarray(inputs['g1_W1'], np.float32),
        w_g1b1=np.asarray(inputs['g1_b1'], np.float32).reshape(HID, 1),
        w_g1w2=np.asarray(inputs['g1_W2'], np.float32),
        w_g1b2=np.asarray(inputs['g1_b2'], np.float32).reshape(1, 1),
        w_g2w1=np.asarray(inputs['g2_W1'], np.float32),
        w_g2b1=np.asarray(inputs['g2_b1'], np.float32).reshape(HID, 1),
        w_g2w2=np.asarray(inputs['g2_W2'], np.float32),
        w_g2b2=np.asarray(inputs['g2_b2'], np.float32).reshape(DIM, 1),
        w_pw1=np.asarray(inputs['p_W1'], np.float32),
        w_pb1=np.asarray(inputs['p_b1'], np.float32).reshape(HID, 1),
        w_pw2=np.asarray(inputs['p_W2'], np.float32),
        w_pb2=np.asarray(inputs['p_b2'], np.float32).reshape(HID, 1),
        w_pw3=np.asarray(inputs['p_W3'], np.float32),
        w_pb3=np.asarray(inputs['p_b3'], np.float32).reshape(1, 1),
    )

    in_maps = []
    for c in range(NC):
        xi = np.full(NPAD, -1.0, np.float32)
        xi[:NPC] = x_idx[c * NPC:(c + 1) * NPC].astype(np.float32)
        mm = dict(shared)
        mm.update(cores[c])
        mm['e_xidx'] = xi
        in_maps.append(mm)
    return M, in_maps


def kernel(**inputs):
    M, in_maps = _prep_host(inputs)
    if M not in _cache:
        _cache[M] = _build(M)
    nc = _cache[M]
    res = run_bass_kernel_spmd(nc, in_maps, core_ids=list(range(NC)))
    out = np.concatenate([np.asarray(res.results[c]['out']).reshape(8)
                          for c in range(NC)])
    return out.astype(np.float32)


if __name__ == "__main__":
    import jax
    sys.path.insert(0, '/root/problem')
    import reference as R
    with jax.default_device(jax.devices('cpu')[0]):
        inp = R.setup_inputs()
        exp = np.asarray(R.reference(**inp))
    inp = {k: np.asarray(v) for k, v in inp.items()}
    act = kernel(**inp)
    rel = np.linalg.norm(act - exp) / np.linalg.norm(exp)
    print("Relative error:", rel)

